# revision 8
# baseline (speedup 1.0000x reference)
"""MixHop GNN Bass kernel for 8 Trainium2 NeuronCores.

Self-contained: host-side preprocessing (numpy) + Bass/Tile device kernel.

Algorithm notes
---------------
Reference computes, per layer l (widths D_l = 128, 180, 180):
    P0 = X W0^T + b0
    P1 = A (X W1^T) + b1          (A = D^-1/2 (Adj + I) D^-1/2, GCN norm)
    P2 = A^2 (X W2^T) + b2
    X' = BN([P0 | P1 | P2])       (eval-mode affine)
then OUT = X4 linW^T + lin_b.

We fold BN into the weights/biases, and factorize A = S Adjhat S with
S = diag(dis), dis = 1/sqrt(deg).  So each hop is a *pure* gather +
segment-sum of unscaled rows plus cheap per-node scalings.

Device data flow per layer (per core; nodes sharded 6250/core):
    1. PE: Y = X_loc @ U   (U columns: [W1'(60)|pad|W2'(60)|pad|W0'(60)|pad], 192 wide)
    2. T12 = dis * Y[:, 0:128]; DMA -> stage; AllGather -> YT table [50176, 128]
    3. SpMM1: dma_gather YT rows per edge (dest-sorted blocks) -> DVE strided
       tensor_reduce per 128-dest block -> R [6250, 128]; R += T12 (self loop)
    4. Z2' = dis^2 * R[:, 64:124]; DMA; AllGather -> ZT [50176, 64]
    5. SpMM2: gather ZT, reduce -> R2 [6250, 64]; R2 += Z2' (self loop)
    6. X' cols: 0:60 = Y[:,128:188]+c0; 60:120 = dis*R[:,0:60]+c1;
       120:180 = dis*R2[:,0:60]+c2; PE-transpose -> X_T for next layer.

Edges are split in two passes by source table-row half (so dma_gather's
int16 indices stay < 32768), destination-sorted into 128-row blocks padded
to the per-block max in-degree (nodes are dealt to cores round-robin by
global degree rank, so block structure is identical across cores -> one
SPMD program).  Padding slots gather a guaranteed-zero row (pad rows have
dis = 0).
"""

import os
import numpy as np

# ---------------------------------------------------------------- problem dims
N_NODES = 50000
N_EDGES = 400000
D_IN = 128
H = 60
N_CLS = 40
EPS = 1e-5
NC = 8  # cores

CH_COLS = 16  # gather chunk: 16 block-columns = 2048 slots
STAGE = int(os.environ.get("MIXHOP_STAGE", "99"))

TRACE = bool(os.environ.get("MIXHOP_TRACE"))
LAST_EXEC_NS = None
LAST_PROFILE = None


# ================================================================ host planning
def _make_plan(edge_index, n_nodes):
    """Degree-based node permutation, per-core edge slot layout, chunk list."""
    row = np.asarray(edge_index[0], dtype=np.int64)
    col = np.asarray(edge_index[1], dtype=np.int64)
    E = row.shape[0]

    deg = np.bincount(col, minlength=n_nodes) + 1  # + self loop
    dis = (1.0 / np.sqrt(deg.astype(np.float64))).astype(np.float32)

    lpc_real = n_nodes // NC
    nblk = -(-lpc_real // 128)
    lpc = nblk * 128
    nrows = NC * lpc
    half = nrows // 2
    assert half <= 32768, (half, "int16 gather index range exceeded")

    # global rank by degree (desc); node at rank g -> core g%NC, local g//NC
    order = np.argsort(-deg, kind="stable")  # perm: rank -> node
    rank = np.empty(n_nodes, dtype=np.int64)
    rank[order] = np.arange(n_nodes)
    core_of = rank % NC
    local_of = rank // NC
    trow = core_of * lpc + local_of  # table row per node

    # per-edge attributes
    e_core = core_of[col]
    e_local = local_of[col]
    e_src = trow[row]
    e_pass = (e_src >= half).astype(np.int64)

    # counts per (core, pass, local dest)
    cnt = np.zeros((NC, 2, lpc), dtype=np.int64)
    np.add.at(cnt, (e_core, e_pass, e_local), 1)

    # shared per-block K (max across cores and dests in block)
    kb = np.zeros((2, nblk), dtype=np.int64)
    for p in range(2):
        kb[p] = cnt[:, p, :].reshape(NC, nblk, 128).max(axis=(0, 2))

    cb = np.zeros((2, nblk), dtype=np.int64)  # column base per (pass, block)
    cb[0, 1:] = np.cumsum(kb[0])[:-1]
    cb[1, 1:] = np.cumsum(kb[1])[:-1]
    s_pass = [int(kb[0].sum() * 128), int(kb[1].sum() * 128)]
    base = [0, s_pass[0]]
    s_tot = s_pass[0] + s_pass[1]
    # round total slots to multiple of 16 columns is automatic (each block is
    # 128-slot aligned); s_tot % 16 == 0 trivially.

    # position of each edge within its (core, pass, local) group
    key = (e_core * 2 + e_pass) * lpc + e_local
    o = np.argsort(key, kind="stable")
    ks = key[o]
    grp_change = np.r_[0, np.flatnonzero(np.diff(ks)) + 1]
    grp_sizes = np.diff(np.r_[grp_change, E])
    pos_sorted = np.arange(E) - np.repeat(grp_change, grp_sizes)
    pos = np.empty(E, dtype=np.int64)
    pos[o] = pos_sorted

    # slot index (within the core's idx array) for each edge
    e_blk = e_local // 128
    e_p128 = e_local % 128
    e_col = cb[e_pass, e_blk] + pos
    e_slot = np.array(base)[e_pass] + e_col * 128 + e_p128

    # local gather index for each edge (within its half)
    e_idx = (e_src - np.where(e_pass == 1, half, 0)).astype(np.int16)

    # dummy (padding) targets: a pad row in each half (zero-staged since dis=0)
    assert lpc_real < lpc, "need at least one pad row per core for dummies"
    dummy_lo = np.int16(lpc_real)  # core 0 pad row, trow < half
    dummy_hi = np.int16((NC // 2) * lpc + lpc_real - half)

    # per-core idx arrays, [128, s_tot//16] int16 (16-part wrap, replicated x8)
    idx_all = np.empty((NC, s_tot), dtype=np.int16)
    idx_all[:, : s_pass[0]] = dummy_lo
    idx_all[:, s_pass[0] :] = dummy_hi
    idx_all[e_core, e_slot] = e_idx
    idx_wrapped = np.empty((NC, 16, s_tot // 16), dtype=np.int16)
    for c in range(NC):
        idx_wrapped[c] = idx_all[c].reshape(-1, 16).T
    idx_in = np.ascontiguousarray(np.tile(idx_wrapped, (1, 8, 1)))

    # chunks: walk (pass, block) cols, split into <= CH_COLS col chunks.
    # chunk = dict(pass, col_start, ncols, pieces=[(blk, coff, w, first)])
    chunks = []
    for p in range(2):
        cur = dict(p=p, col0=int(cb[p, 0]), ncols=0, pieces=[])
        for b in range(nblk):
            rem = int(kb[p, b])
            first = True
            while rem > 0:
                if cur["ncols"] == CH_COLS:
                    chunks.append(cur)
                    cur = dict(
                        p=p, col0=cur["col0"] + cur["ncols"], ncols=0, pieces=[]
                    )
                take = min(rem, CH_COLS - cur["ncols"])
                cur["pieces"].append((b, cur["ncols"], take, first))
                cur["ncols"] += take
                rem -= take
                first = False
        if cur["ncols"]:
            chunks.append(cur)

    # blocks never touched by a reduce-write in a layer (need per-layer zeroing)
    zero_blocks = [b for b in range(nblk) if kb[0, b] == 0 and kb[1, b] == 0]
    lo_blocks = set(b for b in range(nblk) if kb[0, b] > 0)
    # blocks whose first write happens in pass hi
    hi_first_blocks = set(
        b for b in range(nblk) if kb[0, b] == 0 and kb[1, b] > 0
    )

    # per-core dis tiles [128, nblk]: partition p, block b -> local b*128+p
    disb = np.zeros((NC, 128, nblk), dtype=np.float32)
    for c in range(NC):
        d = np.zeros(lpc, dtype=np.float32)
        loc_nodes = order[c::NC]  # node ids at locals 0..lpc_real-1
        d[:lpc_real] = dis[loc_nodes]
        disb[c] = d.reshape(nblk, 128).T

    return dict(
        E=E,
        lpc_real=lpc_real,
        nblk=nblk,
        lpc=lpc,
        nrows=nrows,
        half=half,
        order=order,
        idx_in=idx_in,
        chunks=chunks,
        zero_blocks=zero_blocks,
        hi_first_blocks=hi_first_blocks,
        s_tot=s_tot,
        disb=disb,
        kb=kb,
    )


def _fold_weights(inp):
    """Fold BN into conv weights/biases. Returns per-layer U [Din,192], biases."""
    f = np.float32
    Us, biases = [], []
    for li, (wname, bname, bn) in enumerate(
        [("conv1", None, "bn1"), ("conv2", None, "bn2"), ("conv3", None, "bn3")]
    ):
        W = np.asarray(inp[wname + "_W"], dtype=f)  # [3, 60, Din]
        b = np.asarray(inp[wname + "_b"], dtype=f)  # [3, 60]
        g = np.asarray(inp[bn + "_g"], dtype=f)
        bb = np.asarray(inp[bn + "_b"], dtype=f)
        m = np.asarray(inp[bn + "_m"], dtype=f)
        v = np.asarray(inp[bn + "_v"], dtype=f)
        s = g / np.sqrt(v + EPS)
        t = bb - m * s
        s3 = s.reshape(3, H)
        t3 = t.reshape(3, H)
        Din = W.shape[2]
        U = np.zeros((Din, 192), dtype=f)
        U[:, 0:H] = (W[1] * s3[1][:, None]).T
        U[:, 64 : 64 + H] = (W[2] * s3[2][:, None]).T
        U[:, 128 : 128 + H] = (W[0] * s3[0][:, None]).T
        c0 = b[0] * s3[0] + t3[0]
        c1 = b[1] * s3[1] + t3[1]
        c2 = b[2] * s3[2] + t3[2]
        Us.append(U)
        biases.append((c0, c1, c2))
    return Us, biases


# ================================================================ device build
def _build_nc(plan):
    import concourse.bass as bass
    import concourse.bacc as bacc
    import concourse.mybir as mybir
    import concourse.tile as tile
    from concourse.masks import make_identity

    f32 = mybir.dt.float32
    i16 = mybir.dt.int16
    nblk = plan["nblk"]
    lpc = plan["lpc"]
    nrows = plan["nrows"]
    half = plan["half"]
    s_tot = plan["s_tot"]
    chunks = plan["chunks"]
    AX = mybir.AxisListType.X
    ADD = mybir.AluOpType.add
    MUL = mybir.AluOpType.mult

    nc = bacc.Bacc("TRN2", target_bir_lowering=False, debug=False, num_devices=NC)

    # ---- I/O
    xT = nc.declare_dram_parameter("xT", [128, lpc], f32, isOutput=False)
    idx_d = nc.declare_dram_parameter("idx", [128, s_tot // 16], i16, isOutput=False)
    disb_d = nc.declare_dram_parameter("disb", [128, nblk], f32, isOutput=False)
    dis2b_d = nc.declare_dram_parameter("dis2b", [128, nblk], f32, isOutput=False)
    u_d = [
        nc.declare_dram_parameter("u0", [128, 192], f32, isOutput=False),
        (
            nc.declare_dram_parameter("u1a", [128, 192], f32, isOutput=False),
            nc.declare_dram_parameter("u1b", [64, 192], f32, isOutput=False),
        ),
        (
            nc.declare_dram_parameter("u2a", [128, 192], f32, isOutput=False),
            nc.declare_dram_parameter("u2b", [64, 192], f32, isOutput=False),
        ),
    ]
    bias_d = [
        [
            nc.declare_dram_parameter(f"bias{l}_{k}", [128, 64], f32, isOutput=False)
            for k in range(3)
        ]
        for l in range(3)
    ]
    lwa_d = nc.declare_dram_parameter("lwa", [128, N_CLS], f32, isOutput=False)
    lwb_d = nc.declare_dram_parameter("lwb", [64, N_CLS], f32, isOutput=False)
    blin_d = nc.declare_dram_parameter("blin", [128, N_CLS], f32, isOutput=False)
    out_d = nc.declare_dram_parameter("out", [lpc, N_CLS], f32, isOutput=True)

    # ---- internal DRAM (per layer to keep collective buffers single-writer)
    ystage = [nc.dram_tensor(f"ystage{l}", [lpc, 128], f32) for l in range(3)]
    yt = [
        nc.dram_tensor(f"yt{l}", [nrows, 128], f32, addr_space="Shared")
        for l in range(3)
    ]
    zstage = [nc.dram_tensor(f"zstage{l}", [lpc, 64], f32) for l in range(3)]
    zt = [
        nc.dram_tensor(f"zt{l}", [nrows, 64], f32, addr_space="Shared")
        for l in range(3)
    ]
    rg = [list(range(NC))]

    with tile.TileContext(nc) as tc, \
            tc.tile_pool(name="const", bufs=1) as const, \
            tc.tile_pool(name="big", bufs=1) as big:
        # persistent SBUF state
        XTa = big.tile([128, lpc], f32, tag="XTa")
        XTb = big.tile([64, lpc], f32, tag="XTb")
        T12 = big.tile([128, nblk * 128], f32, tag="T12")
        Y0 = big.tile([128, nblk * 64], f32, tag="Y0")
        R1 = big.tile([128, nblk * 128], f32, tag="R1")
        R2 = big.tile([128, nblk * 64], f32, tag="R2")
        ZL = big.tile([128, nblk * 64], f32, tag="ZL")
        OUTALL = big.tile([128, nblk * N_CLS], f32, tag="OUTALL")
        idx_sb = const.tile([128, s_tot // 16], i16)
        disb_sb = const.tile([128, nblk], f32)
        dis2b_sb = const.tile([128, nblk], f32)
        ident = const.tile([128, 128], f32)
        make_identity(nc, ident)

        nc.sync.dma_start(out=idx_sb[:], in_=idx_d[:])
        nc.sync.dma_start(out=disb_sb[:], in_=disb_d[:])
        nc.sync.dma_start(out=dis2b_sb[:], in_=dis2b_d[:])
        nc.sync.dma_start(out=XTa[:], in_=xT[:])

        u_sb = []
        u0 = const.tile([128, 192], f32)
        nc.sync.dma_start(out=u0[:], in_=u_d[0][:])
        u_sb.append((u0, None))
        for l in (1, 2):
            ua = const.tile([128, 192], f32, tag=f"u{l}a")
            ub = const.tile([64, 192], f32, tag=f"u{l}b")
            nc.sync.dma_start(out=ua[:], in_=u_d[l][0][:])
            nc.sync.dma_start(out=ub[:], in_=u_d[l][1][:])
            u_sb.append((ua, ub))
        bias_sb = []
        for l in range(3):
            row = []
            for k in range(3):
                t = const.tile([128, 64], f32, tag=f"b{l}{k}")
                nc.sync.dma_start(out=t[:], in_=bias_d[l][k][:])
                row.append(t)
            bias_sb.append(row)
        lwa = const.tile([128, N_CLS], f32)
        lwb = const.tile([64, N_CLS], f32)
        blin = const.tile([128, N_CLS], f32)
        nc.sync.dma_start(out=lwa[:], in_=lwa_d[:])
        nc.sync.dma_start(out=lwb[:], in_=lwb_d[:])
        nc.sync.dma_start(out=blin[:], in_=blin_d[:])

        # zero-init accumulators (pad-only blocks rely on this)
        nc.any.memset(R1[:], 0.0)
        nc.any.memset(R2[:], 0.0)
        nc.any.memset(ZL[:], 0.0)

        with (
            tc.tile_pool(name="psum", bufs=2, space="PSUM") as psum,
            tc.tile_pool(name="tpsum", bufs=2, space="PSUM") as tpsum,
            tc.tile_pool(name="g1p", bufs=2) as g1p,
            tc.tile_pool(name="g2p", bufs=2) as g2p,
            tc.tile_pool(name="work", bufs=4) as work,
        ):
            for l in range(3):
                ua, ub = u_sb[l]
                # ---------------- A: dense Y = X @ U ; split T12 / Y0
                for rb in range(nblk):
                    ps = psum.tile([128, 192], f32, tag="ps")
                    sl = slice(rb * 128, (rb + 1) * 128)
                    use_b = ub is not None and STAGE >= 6
                    nc.tensor.matmul(
                        out=ps[:],
                        lhsT=XTa[:, sl],
                        rhs=ua[:],
                        start=True,
                        stop=not use_b,
                    )
                    if use_b:
                        nc.tensor.matmul(
                            out=ps[:], lhsT=XTb[:, sl], rhs=ub[:],
                            start=False, stop=True,
                        )
                    nc.any.tensor_copy(out=T12[:, sl], in_=ps[:, 0:128])
                    nc.any.tensor_copy(
                        out=Y0[:, rb * 64 : (rb + 1) * 64], in_=ps[:, 128:192]
                    )
                # T12 *= dis (per row)  [128, nblk, 128] * [128, nblk, 1]
                nc.vector.tensor_tensor(
                    out=T12[:].rearrange("p (b e) -> p b e", e=128),
                    in0=T12[:].rearrange("p (b e) -> p b e", e=128),
                    in1=disb_sb[:].unsqueeze(2).to_broadcast([128, nblk, 128]),
                    op=MUL,
                )
                if STAGE < 2:
                    continue
                nc.sync.dma_start(
                    out=ystage[l][:].rearrange("(b p) e -> p b e", p=128),
                    in_=T12[:].rearrange("p (b e) -> p b e", e=128),
                )
                nc.gpsimd.collective_compute(
                    "AllGather", mybir.AluOpType.bypass,
                    replica_groups=rg, ins=[ystage[l][:]], outs=[yt[l][:]],
                )
                # per-layer zero of never-written blocks
                for b in plan["zero_blocks"]:
                    nc.any.memset(R1[:, b * 128 : (b + 1) * 128], 0.0)
                    nc.any.memset(R2[:, b * 64 : (b + 1) * 64], 0.0)

                # ---------------- B: SpMM1 (gather YT, reduce per block)
                if STAGE < 3:
                    continue
                for ch in chunks:
                    ncols = ch["ncols"]
                    slot0 = (0 if ch["p"] == 0 else plan["kb"][0].sum() * 128) + ch[
                        "col0"
                    ] * 128
                    gt = g1p.tile([128, CH_COLS, 128], f32, tag="g1")
                    src = yt[l][0:half, :] if ch["p"] == 0 else yt[l][half:nrows, :]
                    nc.gpsimd.dma_gather(
                        out_ap=gt[:, :ncols, :],
                        in_ap=src,
                        idxs_ap=idx_sb[:, slot0 // 16 : (slot0 + ncols * 128) // 16],
                        num_idxs=ncols * 128,
                        num_idxs_reg=ncols * 128,
                        elem_size=128,
                        single_packet=False,
                    )
                    for (b, coff, w, first) in ch["pieces"]:
                        first_write = first and (
                            ch["p"] == 0 or b in plan["hi_first_blocks"]
                        )
                        view = gt[:, coff : coff + w, :].rearrange("p c e -> p e c")
                        bsl = slice(b * 128, (b + 1) * 128)
                        if first_write:
                            nc.vector.tensor_reduce(
                                out=R1[:, bsl], in_=view, axis=AX, op=ADD
                            )
                        else:
                            tmp = work.tile([128, 128], f32, tag="t1")
                            nc.vector.tensor_reduce(
                                out=tmp[:], in_=view, axis=AX, op=ADD
                            )
                            nc.any.tensor_tensor(
                                out=R1[:, bsl], in0=R1[:, bsl], in1=tmp[:], op=ADD
                            )
                # self loop: R1 += T12
                if STAGE < 4:
                    continue
                nc.vector.tensor_tensor(out=R1[:], in0=R1[:], in1=T12[:], op=ADD)
                # Z2' = dis^2 * R1[:, 64:124] -> ZL (cols 0:60 of each 64-block)
                r1v = R1[:].rearrange("p (b e) -> p b e", e=128)
                zlv = ZL[:].rearrange("p (b e) -> p b e", e=64)
                nc.vector.tensor_tensor(
                    out=zlv[:, :, 0:60],
                    in0=r1v[:, :, 64:124],
                    in1=dis2b_sb[:].unsqueeze(2).to_broadcast([128, nblk, 60]),
                    op=MUL,
                )
                nc.sync.dma_start(
                    out=zstage[l][:].rearrange("(b p) e -> p b e", p=128),
                    in_=zlv,
                )
                nc.gpsimd.collective_compute(
                    "AllGather", mybir.AluOpType.bypass,
                    replica_groups=rg, ins=[zstage[l][:]], outs=[zt[l][:]],
                )

                # ---------------- C: SpMM2 (gather ZT, reduce)
                if STAGE < 5:
                    continue
                for ch in chunks:
                    ncols = ch["ncols"]
                    slot0 = (0 if ch["p"] == 0 else plan["kb"][0].sum() * 128) + ch[
                        "col0"
                    ] * 128
                    gt = g2p.tile([128, CH_COLS, 64], f32, tag="g2")
                    src = zt[l][0:half, :] if ch["p"] == 0 else zt[l][half:nrows, :]
                    nc.gpsimd.dma_gather(
                        out_ap=gt[:, :ncols, :],
                        in_ap=src,
                        idxs_ap=idx_sb[:, slot0 // 16 : (slot0 + ncols * 128) // 16],
                        num_idxs=ncols * 128,
                        num_idxs_reg=ncols * 128,
                        elem_size=64,
                        single_packet=False,
                    )
                    for (b, coff, w, first) in ch["pieces"]:
                        first_write = first and (
                            ch["p"] == 0 or b in plan["hi_first_blocks"]
                        )
                        view = gt[:, coff : coff + w, :].rearrange("p c e -> p e c")
                        bsl = slice(b * 64, (b + 1) * 64)
                        if first_write:
                            nc.vector.tensor_reduce(
                                out=R2[:, bsl], in_=view, axis=AX, op=ADD
                            )
                        else:
                            tmp = work.tile([128, 64], f32, tag="t2")
                            nc.vector.tensor_reduce(
                                out=tmp[:], in_=view, axis=AX, op=ADD
                            )
                            nc.any.tensor_tensor(
                                out=R2[:, bsl], in0=R2[:, bsl], in1=tmp[:], op=ADD
                            )
                # self loop: R2 += ZL
                nc.vector.tensor_tensor(out=R2[:], in0=R2[:], in1=ZL[:], op=ADD)

                # ---------------- D: assemble X' and transpose to X_T
                if STAGE < 6:
                    continue
                b0, b1, b2 = bias_sb[l]
                r2v = R2[:].rearrange("p (b e) -> p b e", e=64)
                y0v = Y0[:].rearrange("p (b e) -> p b e", e=64)
                for rb in range(nblk):
                    xn = work.tile([128, 192], f32, tag="xn")
                    nc.any.memset(xn[:, 180:192], 0.0)
                    # P0
                    nc.any.tensor_tensor(
                        out=xn[:, 0:60], in0=y0v[:, rb, 0:60],
                        in1=b0[:, 0:60], op=ADD,
                    )
                    # P1 = dis * R1 + c1
                    nc.any.tensor_scalar(
                        out=xn[:, 60:120],
                        in0=r1v[:, rb, 0:60],
                        scalar1=disb_sb[:, rb : rb + 1],
                        scalar2=None,
                        op0=MUL,
                    )
                    nc.any.tensor_tensor(
                        out=xn[:, 60:120], in0=xn[:, 60:120],
                        in1=b1[:, 0:60], op=ADD,
                    )
                    # P2 = dis * R2 + c2
                    nc.any.tensor_scalar(
                        out=xn[:, 120:180],
                        in0=r2v[:, rb, 0:60],
                        scalar1=disb_sb[:, rb : rb + 1],
                        scalar2=None,
                        op0=MUL,
                    )
                    nc.any.tensor_tensor(
                        out=xn[:, 120:180], in0=xn[:, 120:180],
                        in1=b2[:, 0:60], op=ADD,
                    )
                    sl = slice(rb * 128, (rb + 1) * 128)
                    pt = tpsum.tile([128, 128], f32, tag="pt")
                    nc.tensor.transpose(out=pt[:], in_=xn[:, 0:128], identity=ident[:])
                    nc.any.tensor_copy(out=XTa[:, sl], in_=pt[:])
                    pt2 = tpsum.tile([64, 128], f32, tag="pt2")
                    nc.tensor.transpose(
                        out=pt2[:], in_=xn[:, 128:192], identity=ident[:]
                    )
                    nc.any.tensor_copy(out=XTb[:, sl], in_=pt2[:])

            # ---------------- final linear
            for rb in range(nblk):
                sl = slice(rb * 128, (rb + 1) * 128)
                ps = psum.tile([128, N_CLS], f32, tag="pf")
                nc.tensor.matmul(
                    out=ps[:], lhsT=XTa[:, sl], rhs=lwa[:], start=True,
                    stop=STAGE < 6,
                )
                if STAGE >= 6:
                    nc.tensor.matmul(
                        out=ps[:], lhsT=XTb[:, sl], rhs=lwb[:], start=False, stop=True
                    )
                nc.any.tensor_tensor(
                    out=OUTALL[:, rb * N_CLS : (rb + 1) * N_CLS],
                    in0=ps[:], in1=blin[:], op=ADD,
                )
            nc.sync.dma_start(
                out=out_d[:].rearrange("(b p) c -> p b c", p=128),
                in_=OUTALL[:].rearrange("p (b c) -> p b c", c=N_CLS),
            )

    nc.compile()
    return nc


# ================================================================ entry point
def _prepare_inputs(inputs, plan):
    """Build per-core in_maps."""
    f = np.float32
    x = np.asarray(inputs["x"], dtype=f)
    order = plan["order"]
    lpc, lpc_real, nblk = plan["lpc"], plan["lpc_real"], plan["nblk"]
    Us, biases = _fold_weights(inputs)
    lin_W = np.asarray(inputs["lin_W"], dtype=f)  # [40, 180]
    lin_b = np.asarray(inputs["lin_b"], dtype=f)
    lwT = np.zeros((192, N_CLS), dtype=f)
    lwT[0:180, :] = lin_W.T
    blin = np.tile(lin_b[None, :], (128, 1)).astype(f)

    def repl_bias(c):
        t = np.zeros((128, 64), dtype=f)
        t[:, 0:60] = c[None, :]
        return t

    in_maps = []
    for c in range(NC):
        m = {}
        xs = np.zeros((lpc, D_IN), dtype=f)
        xs[:lpc_real] = x[order[c::NC]]
        m["xT"] = np.ascontiguousarray(xs.T)
        m["idx"] = plan["idx_in"][c]
        m["disb"] = np.ascontiguousarray(plan["disb"][c])
        m["dis2b"] = np.ascontiguousarray(plan["disb"][c] ** 2)
        m["u0"] = Us[0]
        m["u1a"] = np.ascontiguousarray(Us[1][0:128])
        u1b = np.zeros((64, 192), dtype=f)
        u1b[0:52] = Us[1][128:180]
        m["u1b"] = u1b
        m["u2a"] = np.ascontiguousarray(Us[2][0:128])
        u2b = np.zeros((64, 192), dtype=f)
        u2b[0:52] = Us[2][128:180]
        m["u2b"] = u2b
        for l in range(3):
            for k in range(3):
                m[f"bias{l}_{k}"] = repl_bias(biases[l][k])
        m["lwa"] = np.ascontiguousarray(lwT[0:128])
        m["lwb"] = np.ascontiguousarray(lwT[128:192])
        m["blin"] = blin
        in_maps.append(m)
    return in_maps


_CACHE = {}


def _install_ntff_shim():
    """Provide antenv.axon_hooks (missing in this image) so trace=True works."""
    import sys, types, ctypes, contextlib

    try:
        from antenv.axon_hooks import get_axon_ntff_profile_hook  # noqa: F401

        return
    except ImportError:
        pass
    so_path = "/opt/axon/libaxon_pjrt.so"
    hook = None
    try:
        lib = ctypes.CDLL(so_path)
        if hasattr(lib, "axon_start_nrt_profile"):
            lib.axon_start_nrt_profile.argtypes = [
                ctypes.POINTER(ctypes.c_int64),
                ctypes.c_size_t,
            ]
            lib.axon_start_nrt_profile.restype = ctypes.c_int64
            lib.axon_stop_nrt_profile.argtypes = [ctypes.c_char_p]
            lib.axon_stop_nrt_profile.restype = ctypes.c_int64

            @contextlib.contextmanager
            def hook(output_dir, device_ids):
                import jax

                jax.devices()
                if device_ids:
                    ids = (ctypes.c_int64 * len(device_ids))(*device_ids)
                    rc = lib.axon_start_nrt_profile(ids, len(device_ids))
                else:
                    rc = lib.axon_start_nrt_profile(None, 0)
                if rc != 0:
                    raise RuntimeError(f"axon_start_nrt_profile rc={rc}")
                try:
                    yield
                finally:
                    n = lib.axon_stop_nrt_profile(str(output_dir).encode())
                    print(f"profile: {n} file(s) written to {output_dir}")

    except OSError:
        pass
    mod = types.ModuleType("antenv.axon_hooks")
    mod.get_axon_ntff_profile_hook = lambda: hook
    mod.set_axon_ntff_profile_hook = lambda h: None
    sys.modules["antenv.axon_hooks"] = mod


def kernel(**inputs):
    global LAST_EXEC_NS, LAST_PROFILE
    from concourse import bass_utils

    if TRACE:
        _install_ntff_shim()
        bass_utils.upload_artifacts = lambda tmpdir: tmpdir

    edge_index = np.asarray(inputs["edge_index"])
    key = ("plan", edge_index.shape[1])
    if key not in _CACHE:
        plan = _make_plan(edge_index, N_NODES)
        nc = _build_nc(plan)
        _CACHE[key] = (plan, nc)
    plan, nc = _CACHE[key]

    in_maps = _prepare_inputs(inputs, plan)
    if os.environ.get("MIXHOP_SIM"):
        from concourse import bass_interp

        sim = bass_interp.MultiCoreSim(nc, NC, num_workers=NC)
        for c in range(NC):
            for k, v in in_maps[c].items():
                sim.cores[c].tensor(k)[:] = v
        sim.simulate()
        outs = [{"out": np.array(sim.cores[c].mem_tensor("out"))} for c in range(NC)]
    else:
        res = bass_utils.run_bass_kernel_spmd(
            nc, in_maps, core_ids=list(range(NC)), trace=TRACE
        )
        LAST_EXEC_NS = res.exec_time_ns
        LAST_PROFILE = res.profile_json
        outs = res.results

    lpc, lpc_real = plan["lpc"], plan["lpc_real"]
    order = plan["order"]
    full = np.empty((N_NODES, N_CLS), dtype=np.float32)
    for c in range(NC):
        full[order[c::NC]] = outs[c]["out"][:lpc_real]
    return full


# revision 9
# speedup vs baseline: 1.0858x; 1.0858x over previous
"""MixHop GNN Bass kernel for 8 Trainium2 NeuronCores.

Self-contained: host-side preprocessing (numpy) + Bass/Tile device kernel.

Algorithm notes
---------------
Reference computes, per layer l (widths D_l = 128, 180, 180):
    P0 = X W0^T + b0
    P1 = A (X W1^T) + b1          (A = D^-1/2 (Adj + I) D^-1/2, GCN norm)
    P2 = A^2 (X W2^T) + b2
    X' = BN([P0 | P1 | P2])       (eval-mode affine)
then OUT = X4 linW^T + lin_b.

We fold BN into the weights/biases, and factorize A = S Adjhat S with
S = diag(dis), dis = 1/sqrt(deg).  So each hop is a *pure* gather +
segment-sum of unscaled rows plus cheap per-node scalings.

Device data flow per layer (per core; nodes sharded 6250/core):
    1. PE: Y = X_loc @ U   (U columns: [W1'(60)|pad|W2'(60)|pad|W0'(60)|pad], 192 wide)
    2. T12 = dis * Y[:, 0:128]; DMA -> stage; AllGather -> YT table [50176, 128]
    3. SpMM1: dma_gather YT rows per edge (dest-sorted blocks) -> DVE strided
       tensor_reduce per 128-dest block -> R [6250, 128]; R += T12 (self loop)
    4. Z2' = dis^2 * R[:, 64:124]; DMA; AllGather -> ZT [50176, 64]
    5. SpMM2: gather ZT, reduce -> R2 [6250, 64]; R2 += Z2' (self loop)
    6. X' cols: 0:60 = Y[:,128:188]+c0; 60:120 = dis*R[:,0:60]+c1;
       120:180 = dis*R2[:,0:60]+c2; PE-transpose -> X_T for next layer.

Edges are split in two passes by source table-row half (so dma_gather's
int16 indices stay < 32768), destination-sorted into 128-row blocks padded
to the per-block max in-degree (nodes are dealt to cores round-robin by
global degree rank, so block structure is identical across cores -> one
SPMD program).  Padding slots gather a guaranteed-zero row (pad rows have
dis = 0).
"""

import os
import numpy as np

# ---------------------------------------------------------------- problem dims
N_NODES = 50000
N_EDGES = 400000
D_IN = 128
H = 60
N_CLS = 40
EPS = 1e-5
NC = 8  # cores

CH_COLS = 8  # gather chunk: 8 block-columns = 1024 slots (single_packet limit)
STAGE = int(os.environ.get("MIXHOP_STAGE", "99"))

TRACE = bool(os.environ.get("MIXHOP_TRACE"))
LAST_EXEC_NS = None
LAST_PROFILE = None


# ================================================================ host planning
def _make_plan(edge_index, n_nodes):
    """Degree-based node permutation, per-core edge slot layout, chunk list."""
    row = np.asarray(edge_index[0], dtype=np.int64)
    col = np.asarray(edge_index[1], dtype=np.int64)
    E = row.shape[0]

    deg = np.bincount(col, minlength=n_nodes) + 1  # + self loop
    dis = (1.0 / np.sqrt(deg.astype(np.float64))).astype(np.float32)

    lpc_real = n_nodes // NC
    nblk = -(-lpc_real // 128)
    lpc = nblk * 128
    nrows = NC * lpc
    half = nrows // 2
    assert half <= 32768, (half, "int16 gather index range exceeded")

    # global rank by degree (desc); node at rank g -> core g%NC, local g//NC
    order = np.argsort(-deg, kind="stable")  # perm: rank -> node
    rank = np.empty(n_nodes, dtype=np.int64)
    rank[order] = np.arange(n_nodes)
    core_of = rank % NC
    local_of = rank // NC
    trow = core_of * lpc + local_of  # table row per node

    # per-edge attributes
    e_core = core_of[col]
    e_local = local_of[col]
    e_src = trow[row]
    e_pass = (e_src >= half).astype(np.int64)

    # counts per (core, pass, local dest)
    cnt = np.zeros((NC, 2, lpc), dtype=np.int64)
    np.add.at(cnt, (e_core, e_pass, e_local), 1)

    # shared per-block K (max across cores and dests in block)
    kb = np.zeros((2, nblk), dtype=np.int64)
    for p in range(2):
        kb[p] = cnt[:, p, :].reshape(NC, nblk, 128).max(axis=(0, 2))

    cb = np.zeros((2, nblk), dtype=np.int64)  # column base per (pass, block)
    cb[0, 1:] = np.cumsum(kb[0])[:-1]
    cb[1, 1:] = np.cumsum(kb[1])[:-1]
    s_pass = [int(kb[0].sum() * 128), int(kb[1].sum() * 128)]
    base = [0, s_pass[0]]
    s_tot = s_pass[0] + s_pass[1]
    # round total slots to multiple of 16 columns is automatic (each block is
    # 128-slot aligned); s_tot % 16 == 0 trivially.

    # position of each edge within its (core, pass, local) group
    key = (e_core * 2 + e_pass) * lpc + e_local
    o = np.argsort(key, kind="stable")
    ks = key[o]
    grp_change = np.r_[0, np.flatnonzero(np.diff(ks)) + 1]
    grp_sizes = np.diff(np.r_[grp_change, E])
    pos_sorted = np.arange(E) - np.repeat(grp_change, grp_sizes)
    pos = np.empty(E, dtype=np.int64)
    pos[o] = pos_sorted

    # slot index (within the core's idx array) for each edge
    e_blk = e_local // 128
    e_p128 = e_local % 128
    e_col = cb[e_pass, e_blk] + pos
    e_slot = np.array(base)[e_pass] + e_col * 128 + e_p128

    # local gather index for each edge (within its half)
    e_idx = (e_src - np.where(e_pass == 1, half, 0)).astype(np.int16)

    # dummy (padding) targets: a pad row in each half (zero-staged since dis=0)
    assert lpc_real < lpc, "need at least one pad row per core for dummies"
    dummy_lo = np.int16(lpc_real)  # core 0 pad row, trow < half
    dummy_hi = np.int16((NC // 2) * lpc + lpc_real - half)

    # per-core idx arrays, [128, s_tot//16] int16 (16-part wrap, replicated x8)
    idx_all = np.empty((NC, s_tot), dtype=np.int16)
    idx_all[:, : s_pass[0]] = dummy_lo
    idx_all[:, s_pass[0] :] = dummy_hi
    idx_all[e_core, e_slot] = e_idx
    idx_wrapped = np.empty((NC, 16, s_tot // 16), dtype=np.int16)
    for c in range(NC):
        idx_wrapped[c] = idx_all[c].reshape(-1, 16).T
    idx_in = np.ascontiguousarray(np.tile(idx_wrapped, (1, 8, 1)))

    # chunks: walk (pass, block) cols, split into <= CH_COLS col chunks.
    # chunk = dict(pass, col_start, ncols, pieces=[(blk, coff, w, first)])
    chunks = []
    for p in range(2):
        cur = dict(p=p, col0=int(cb[p, 0]), ncols=0, pieces=[])
        for b in range(nblk):
            rem = int(kb[p, b])
            first = True
            while rem > 0:
                if cur["ncols"] == CH_COLS:
                    chunks.append(cur)
                    cur = dict(
                        p=p, col0=cur["col0"] + cur["ncols"], ncols=0, pieces=[]
                    )
                take = min(rem, CH_COLS - cur["ncols"])
                cur["pieces"].append((b, cur["ncols"], take, first))
                cur["ncols"] += take
                rem -= take
                first = False
        if cur["ncols"]:
            chunks.append(cur)

    # blocks never touched by a reduce-write in a layer (need per-layer zeroing)
    zero_blocks = [b for b in range(nblk) if kb[0, b] == 0 and kb[1, b] == 0]
    lo_blocks = set(b for b in range(nblk) if kb[0, b] > 0)
    # blocks whose first write happens in pass hi
    hi_first_blocks = set(
        b for b in range(nblk) if kb[0, b] == 0 and kb[1, b] > 0
    )

    # per-core dis tiles [128, nblk]: partition p, block b -> local b*128+p
    disb = np.zeros((NC, 128, nblk), dtype=np.float32)
    for c in range(NC):
        d = np.zeros(lpc, dtype=np.float32)
        loc_nodes = order[c::NC]  # node ids at locals 0..lpc_real-1
        d[:lpc_real] = dis[loc_nodes]
        disb[c] = d.reshape(nblk, 128).T

    return dict(
        E=E,
        lpc_real=lpc_real,
        nblk=nblk,
        lpc=lpc,
        nrows=nrows,
        half=half,
        order=order,
        idx_in=idx_in,
        chunks=chunks,
        zero_blocks=zero_blocks,
        hi_first_blocks=hi_first_blocks,
        s_tot=s_tot,
        disb=disb,
        kb=kb,
    )


def _fold_weights(inp):
    """Fold BN into conv weights/biases. Returns per-layer U [Din,192], biases."""
    f = np.float32
    Us, biases = [], []
    for li, (wname, bname, bn) in enumerate(
        [("conv1", None, "bn1"), ("conv2", None, "bn2"), ("conv3", None, "bn3")]
    ):
        W = np.asarray(inp[wname + "_W"], dtype=f)  # [3, 60, Din]
        b = np.asarray(inp[wname + "_b"], dtype=f)  # [3, 60]
        g = np.asarray(inp[bn + "_g"], dtype=f)
        bb = np.asarray(inp[bn + "_b"], dtype=f)
        m = np.asarray(inp[bn + "_m"], dtype=f)
        v = np.asarray(inp[bn + "_v"], dtype=f)
        s = g / np.sqrt(v + EPS)
        t = bb - m * s
        s3 = s.reshape(3, H)
        t3 = t.reshape(3, H)
        Din = W.shape[2]
        U = np.zeros((Din, 192), dtype=f)
        U[:, 0:H] = (W[1] * s3[1][:, None]).T
        U[:, 64 : 64 + H] = (W[2] * s3[2][:, None]).T
        U[:, 128 : 128 + H] = (W[0] * s3[0][:, None]).T
        c0 = b[0] * s3[0] + t3[0]
        c1 = b[1] * s3[1] + t3[1]
        c2 = b[2] * s3[2] + t3[2]
        Us.append(U)
        biases.append((c0, c1, c2))
    return Us, biases


# ================================================================ device build
def _build_nc(plan):
    import concourse.bass as bass
    import concourse.bacc as bacc
    import concourse.mybir as mybir
    import concourse.tile as tile
    from concourse.masks import make_identity

    f32 = mybir.dt.float32
    i16 = mybir.dt.int16
    nblk = plan["nblk"]
    lpc = plan["lpc"]
    nrows = plan["nrows"]
    half = plan["half"]
    s_tot = plan["s_tot"]
    chunks = plan["chunks"]
    AX = mybir.AxisListType.X
    ADD = mybir.AluOpType.add
    MUL = mybir.AluOpType.mult

    nc = bacc.Bacc("TRN2", target_bir_lowering=False, debug=False, num_devices=NC)

    # ---- I/O
    xT = nc.declare_dram_parameter("xT", [128, lpc], f32, isOutput=False)
    idx_d = nc.declare_dram_parameter("idx", [128, s_tot // 16], i16, isOutput=False)
    disb_d = nc.declare_dram_parameter("disb", [128, nblk], f32, isOutput=False)
    dis2b_d = nc.declare_dram_parameter("dis2b", [128, nblk], f32, isOutput=False)
    u_d = [
        nc.declare_dram_parameter("u0", [128, 192], f32, isOutput=False),
        (
            nc.declare_dram_parameter("u1a", [128, 192], f32, isOutput=False),
            nc.declare_dram_parameter("u1b", [64, 192], f32, isOutput=False),
        ),
        (
            nc.declare_dram_parameter("u2a", [128, 192], f32, isOutput=False),
            nc.declare_dram_parameter("u2b", [64, 192], f32, isOutput=False),
        ),
    ]
    bias_d = [
        [
            nc.declare_dram_parameter(f"bias{l}_{k}", [128, 64], f32, isOutput=False)
            for k in range(3)
        ]
        for l in range(3)
    ]
    lwa_d = nc.declare_dram_parameter("lwa", [128, N_CLS], f32, isOutput=False)
    lwb_d = nc.declare_dram_parameter("lwb", [64, N_CLS], f32, isOutput=False)
    blin_d = nc.declare_dram_parameter("blin", [128, N_CLS], f32, isOutput=False)
    out_d = nc.declare_dram_parameter("out", [lpc, N_CLS], f32, isOutput=True)

    # ---- internal DRAM (per layer to keep collective buffers single-writer)
    ystage = [nc.dram_tensor(f"ystage{l}", [lpc, 128], f32) for l in range(3)]
    yt = [
        nc.dram_tensor(f"yt{l}", [nrows, 128], f32, addr_space="Shared")
        for l in range(3)
    ]
    zstage = [nc.dram_tensor(f"zstage{l}", [lpc, 64], f32) for l in range(3)]
    zt = [
        nc.dram_tensor(f"zt{l}", [nrows, 64], f32, addr_space="Shared")
        for l in range(3)
    ]
    rg = [list(range(NC))]

    with tile.TileContext(nc) as tc, \
            tc.tile_pool(name="const", bufs=1) as const, \
            tc.tile_pool(name="big", bufs=1) as big:
        # persistent SBUF state
        XTa = big.tile([128, lpc], f32, tag="XTa")
        XTb = big.tile([64, lpc], f32, tag="XTb")
        T12 = big.tile([128, nblk * 128], f32, tag="T12")
        Y0 = big.tile([128, nblk * 64], f32, tag="Y0")
        R1 = big.tile([128, nblk * 128], f32, tag="R1")
        R2 = big.tile([128, nblk * 64], f32, tag="R2")
        ZL = big.tile([128, nblk * 64], f32, tag="ZL")
        OUTALL = big.tile([128, nblk * N_CLS], f32, tag="OUTALL")
        idx_sb = const.tile([128, s_tot // 16], i16)
        disb_sb = const.tile([128, nblk], f32)
        dis2b_sb = const.tile([128, nblk], f32)
        ident = const.tile([128, 128], f32)
        make_identity(nc, ident)

        nc.sync.dma_start(out=idx_sb[:], in_=idx_d[:])
        nc.sync.dma_start(out=disb_sb[:], in_=disb_d[:])
        nc.sync.dma_start(out=dis2b_sb[:], in_=dis2b_d[:])
        nc.sync.dma_start(out=XTa[:], in_=xT[:])

        u_sb = []
        u0 = const.tile([128, 192], f32)
        nc.sync.dma_start(out=u0[:], in_=u_d[0][:])
        u_sb.append((u0, None))
        for l in (1, 2):
            ua = const.tile([128, 192], f32, tag=f"u{l}a")
            ub = const.tile([64, 192], f32, tag=f"u{l}b")
            nc.sync.dma_start(out=ua[:], in_=u_d[l][0][:])
            nc.sync.dma_start(out=ub[:], in_=u_d[l][1][:])
            u_sb.append((ua, ub))
        bias_sb = []
        for l in range(3):
            row = []
            for k in range(3):
                t = const.tile([128, 64], f32, tag=f"b{l}{k}")
                nc.sync.dma_start(out=t[:], in_=bias_d[l][k][:])
                row.append(t)
            bias_sb.append(row)
        lwa = const.tile([128, N_CLS], f32)
        lwb = const.tile([64, N_CLS], f32)
        blin = const.tile([128, N_CLS], f32)
        nc.sync.dma_start(out=lwa[:], in_=lwa_d[:])
        nc.sync.dma_start(out=lwb[:], in_=lwb_d[:])
        nc.sync.dma_start(out=blin[:], in_=blin_d[:])

        # zero-init accumulators (pad-only blocks rely on this)
        nc.any.memset(R1[:], 0.0)
        nc.any.memset(R2[:], 0.0)
        nc.any.memset(ZL[:], 0.0)

        with (
            tc.tile_pool(name="psum", bufs=2, space="PSUM") as psum,
            tc.tile_pool(name="tpsum", bufs=2, space="PSUM") as tpsum,
            tc.tile_pool(name="g1p", bufs=3) as g1p,
            tc.tile_pool(name="g2p", bufs=3) as g2p,
            tc.tile_pool(name="work", bufs=4) as work,
        ):
            for l in range(3):
                ua, ub = u_sb[l]
                # ---------------- A: dense Y = X @ U ; split T12 / Y0
                for rb in range(nblk):
                    ps = psum.tile([128, 192], f32, tag="ps")
                    sl = slice(rb * 128, (rb + 1) * 128)
                    use_b = ub is not None and STAGE >= 6
                    nc.tensor.matmul(
                        out=ps[:],
                        lhsT=XTa[:, sl],
                        rhs=ua[:],
                        start=True,
                        stop=not use_b,
                    )
                    if use_b:
                        nc.tensor.matmul(
                            out=ps[:], lhsT=XTb[:, sl], rhs=ub[:],
                            start=False, stop=True,
                        )
                    nc.any.tensor_copy(out=T12[:, sl], in_=ps[:, 0:128])
                    nc.any.tensor_copy(
                        out=Y0[:, rb * 64 : (rb + 1) * 64], in_=ps[:, 128:192]
                    )
                # T12 *= dis (per row)  [128, nblk, 128] * [128, nblk, 1]
                nc.vector.tensor_tensor(
                    out=T12[:].rearrange("p (b e) -> p b e", e=128),
                    in0=T12[:].rearrange("p (b e) -> p b e", e=128),
                    in1=disb_sb[:].unsqueeze(2).to_broadcast([128, nblk, 128]),
                    op=MUL,
                )
                if STAGE < 2:
                    continue
                nc.sync.dma_start(
                    out=ystage[l][:].rearrange("(b p) e -> p b e", p=128),
                    in_=T12[:].rearrange("p (b e) -> p b e", e=128),
                )
                nc.gpsimd.collective_compute(
                    "AllGather", mybir.AluOpType.bypass,
                    replica_groups=rg, ins=[ystage[l][:]], outs=[yt[l][:]],
                )
                # per-layer zero of never-written blocks
                for b in plan["zero_blocks"]:
                    nc.any.memset(R1[:, b * 128 : (b + 1) * 128], 0.0)
                    nc.any.memset(R2[:, b * 64 : (b + 1) * 64], 0.0)

                # ---------------- B: SpMM1 (gather YT, reduce per block)
                if STAGE < 3:
                    continue
                for ch in chunks:
                    ncols = ch["ncols"]
                    slot0 = (0 if ch["p"] == 0 else plan["kb"][0].sum() * 128) + ch[
                        "col0"
                    ] * 128
                    gt = g1p.tile([128, CH_COLS, 128], f32, tag="g1")
                    src = yt[l][0:half, :] if ch["p"] == 0 else yt[l][half:nrows, :]
                    nc.gpsimd.dma_gather(
                        out_ap=gt[:, :ncols, :],
                        in_ap=src,
                        idxs_ap=idx_sb[:, slot0 // 16 : (slot0 + ncols * 128) // 16],
                        num_idxs=ncols * 128,
                        num_idxs_reg=ncols * 128,
                        elem_size=128,
                    )
                    for (b, coff, w, first) in ch["pieces"]:
                        first_write = first and (
                            ch["p"] == 0 or b in plan["hi_first_blocks"]
                        )
                        view = gt[:, coff : coff + w, :].rearrange("p c e -> p e c")
                        bsl = slice(b * 128, (b + 1) * 128)
                        if first_write:
                            nc.vector.tensor_reduce(
                                out=R1[:, bsl], in_=view, axis=AX, op=ADD
                            )
                        else:
                            tmp = work.tile([128, 128], f32, tag="t1")
                            nc.vector.tensor_reduce(
                                out=tmp[:], in_=view, axis=AX, op=ADD
                            )
                            nc.any.tensor_tensor(
                                out=R1[:, bsl], in0=R1[:, bsl], in1=tmp[:], op=ADD
                            )
                # self loop: R1 += T12
                if STAGE < 4:
                    continue
                nc.vector.tensor_tensor(out=R1[:], in0=R1[:], in1=T12[:], op=ADD)
                # Z2' = dis^2 * R1[:, 64:124] -> ZL (cols 0:60 of each 64-block)
                r1v = R1[:].rearrange("p (b e) -> p b e", e=128)
                zlv = ZL[:].rearrange("p (b e) -> p b e", e=64)
                nc.vector.tensor_tensor(
                    out=zlv[:, :, 0:60],
                    in0=r1v[:, :, 64:124],
                    in1=dis2b_sb[:].unsqueeze(2).to_broadcast([128, nblk, 60]),
                    op=MUL,
                )
                nc.sync.dma_start(
                    out=zstage[l][:].rearrange("(b p) e -> p b e", p=128),
                    in_=zlv,
                )
                nc.gpsimd.collective_compute(
                    "AllGather", mybir.AluOpType.bypass,
                    replica_groups=rg, ins=[zstage[l][:]], outs=[zt[l][:]],
                )

                # ---------------- C: SpMM2 (gather ZT, reduce)
                if STAGE < 5:
                    continue
                for ch in chunks:
                    ncols = ch["ncols"]
                    slot0 = (0 if ch["p"] == 0 else plan["kb"][0].sum() * 128) + ch[
                        "col0"
                    ] * 128
                    gt = g2p.tile([128, CH_COLS, 64], f32, tag="g2")
                    src = zt[l][0:half, :] if ch["p"] == 0 else zt[l][half:nrows, :]
                    nc.gpsimd.dma_gather(
                        out_ap=gt[:, :ncols, :],
                        in_ap=src,
                        idxs_ap=idx_sb[:, slot0 // 16 : (slot0 + ncols * 128) // 16],
                        num_idxs=ncols * 128,
                        num_idxs_reg=ncols * 128,
                        elem_size=64,
                    )
                    for (b, coff, w, first) in ch["pieces"]:
                        first_write = first and (
                            ch["p"] == 0 or b in plan["hi_first_blocks"]
                        )
                        view = gt[:, coff : coff + w, :].rearrange("p c e -> p e c")
                        bsl = slice(b * 64, (b + 1) * 64)
                        if first_write:
                            nc.vector.tensor_reduce(
                                out=R2[:, bsl], in_=view, axis=AX, op=ADD
                            )
                        else:
                            tmp = work.tile([128, 64], f32, tag="t2")
                            nc.vector.tensor_reduce(
                                out=tmp[:], in_=view, axis=AX, op=ADD
                            )
                            nc.any.tensor_tensor(
                                out=R2[:, bsl], in0=R2[:, bsl], in1=tmp[:], op=ADD
                            )
                # self loop: R2 += ZL
                nc.vector.tensor_tensor(out=R2[:], in0=R2[:], in1=ZL[:], op=ADD)

                # ---------------- D: assemble X' and transpose to X_T
                if STAGE < 6:
                    continue
                b0, b1, b2 = bias_sb[l]
                r2v = R2[:].rearrange("p (b e) -> p b e", e=64)
                y0v = Y0[:].rearrange("p (b e) -> p b e", e=64)
                for rb in range(nblk):
                    xn = work.tile([128, 192], f32, tag="xn")
                    nc.any.memset(xn[:, 180:192], 0.0)
                    # P0
                    nc.any.tensor_tensor(
                        out=xn[:, 0:60], in0=y0v[:, rb, 0:60],
                        in1=b0[:, 0:60], op=ADD,
                    )
                    # P1 = dis * R1 + c1
                    nc.any.tensor_scalar(
                        out=xn[:, 60:120],
                        in0=r1v[:, rb, 0:60],
                        scalar1=disb_sb[:, rb : rb + 1],
                        scalar2=None,
                        op0=MUL,
                    )
                    nc.any.tensor_tensor(
                        out=xn[:, 60:120], in0=xn[:, 60:120],
                        in1=b1[:, 0:60], op=ADD,
                    )
                    # P2 = dis * R2 + c2
                    nc.any.tensor_scalar(
                        out=xn[:, 120:180],
                        in0=r2v[:, rb, 0:60],
                        scalar1=disb_sb[:, rb : rb + 1],
                        scalar2=None,
                        op0=MUL,
                    )
                    nc.any.tensor_tensor(
                        out=xn[:, 120:180], in0=xn[:, 120:180],
                        in1=b2[:, 0:60], op=ADD,
                    )
                    sl = slice(rb * 128, (rb + 1) * 128)
                    pt = tpsum.tile([128, 128], f32, tag="pt")
                    nc.tensor.transpose(out=pt[:], in_=xn[:, 0:128], identity=ident[:])
                    nc.any.tensor_copy(out=XTa[:, sl], in_=pt[:])
                    pt2 = tpsum.tile([64, 128], f32, tag="pt2")
                    nc.tensor.transpose(
                        out=pt2[:], in_=xn[:, 128:192], identity=ident[:]
                    )
                    nc.any.tensor_copy(out=XTb[:, sl], in_=pt2[:])

            # ---------------- final linear
            for rb in range(nblk):
                sl = slice(rb * 128, (rb + 1) * 128)
                ps = psum.tile([128, N_CLS], f32, tag="pf")
                nc.tensor.matmul(
                    out=ps[:], lhsT=XTa[:, sl], rhs=lwa[:], start=True,
                    stop=STAGE < 6,
                )
                if STAGE >= 6:
                    nc.tensor.matmul(
                        out=ps[:], lhsT=XTb[:, sl], rhs=lwb[:], start=False, stop=True
                    )
                nc.any.tensor_tensor(
                    out=OUTALL[:, rb * N_CLS : (rb + 1) * N_CLS],
                    in0=ps[:], in1=blin[:], op=ADD,
                )
            nc.sync.dma_start(
                out=out_d[:].rearrange("(b p) c -> p b c", p=128),
                in_=OUTALL[:].rearrange("p (b c) -> p b c", c=N_CLS),
            )

    nc.compile()
    return nc


# ================================================================ entry point
def _prepare_inputs(inputs, plan):
    """Build per-core in_maps."""
    f = np.float32
    x = np.asarray(inputs["x"], dtype=f)
    order = plan["order"]
    lpc, lpc_real, nblk = plan["lpc"], plan["lpc_real"], plan["nblk"]
    Us, biases = _fold_weights(inputs)
    lin_W = np.asarray(inputs["lin_W"], dtype=f)  # [40, 180]
    lin_b = np.asarray(inputs["lin_b"], dtype=f)
    lwT = np.zeros((192, N_CLS), dtype=f)
    lwT[0:180, :] = lin_W.T
    blin = np.tile(lin_b[None, :], (128, 1)).astype(f)

    def repl_bias(c):
        t = np.zeros((128, 64), dtype=f)
        t[:, 0:60] = c[None, :]
        return t

    in_maps = []
    for c in range(NC):
        m = {}
        xs = np.zeros((lpc, D_IN), dtype=f)
        xs[:lpc_real] = x[order[c::NC]]
        m["xT"] = np.ascontiguousarray(xs.T)
        m["idx"] = plan["idx_in"][c]
        m["disb"] = np.ascontiguousarray(plan["disb"][c])
        m["dis2b"] = np.ascontiguousarray(plan["disb"][c] ** 2)
        m["u0"] = Us[0]
        m["u1a"] = np.ascontiguousarray(Us[1][0:128])
        u1b = np.zeros((64, 192), dtype=f)
        u1b[0:52] = Us[1][128:180]
        m["u1b"] = u1b
        m["u2a"] = np.ascontiguousarray(Us[2][0:128])
        u2b = np.zeros((64, 192), dtype=f)
        u2b[0:52] = Us[2][128:180]
        m["u2b"] = u2b
        for l in range(3):
            for k in range(3):
                m[f"bias{l}_{k}"] = repl_bias(biases[l][k])
        m["lwa"] = np.ascontiguousarray(lwT[0:128])
        m["lwb"] = np.ascontiguousarray(lwT[128:192])
        m["blin"] = blin
        in_maps.append(m)
    return in_maps


_CACHE = {}


def _install_ntff_shim():
    """Provide antenv.axon_hooks (missing in this image) so trace=True works."""
    import sys, types, ctypes, contextlib

    try:
        from antenv.axon_hooks import get_axon_ntff_profile_hook  # noqa: F401

        return
    except ImportError:
        pass
    so_path = "/opt/axon/libaxon_pjrt.so"
    hook = None
    try:
        lib = ctypes.CDLL(so_path)
        if hasattr(lib, "axon_start_nrt_profile"):
            lib.axon_start_nrt_profile.argtypes = [
                ctypes.POINTER(ctypes.c_int64),
                ctypes.c_size_t,
            ]
            lib.axon_start_nrt_profile.restype = ctypes.c_int64
            lib.axon_stop_nrt_profile.argtypes = [ctypes.c_char_p]
            lib.axon_stop_nrt_profile.restype = ctypes.c_int64

            @contextlib.contextmanager
            def hook(output_dir, device_ids):
                import jax

                jax.devices()
                if device_ids:
                    ids = (ctypes.c_int64 * len(device_ids))(*device_ids)
                    rc = lib.axon_start_nrt_profile(ids, len(device_ids))
                else:
                    rc = lib.axon_start_nrt_profile(None, 0)
                if rc != 0:
                    raise RuntimeError(f"axon_start_nrt_profile rc={rc}")
                try:
                    yield
                finally:
                    n = lib.axon_stop_nrt_profile(str(output_dir).encode())
                    print(f"profile: {n} file(s) written to {output_dir}")

    except OSError:
        pass
    mod = types.ModuleType("antenv.axon_hooks")
    mod.get_axon_ntff_profile_hook = lambda: hook
    mod.set_axon_ntff_profile_hook = lambda h: None
    sys.modules["antenv.axon_hooks"] = mod


def kernel(**inputs):
    global LAST_EXEC_NS, LAST_PROFILE
    from concourse import bass_utils

    if TRACE:
        _install_ntff_shim()
        bass_utils.upload_artifacts = lambda tmpdir: tmpdir

    edge_index = np.asarray(inputs["edge_index"])
    key = ("plan", edge_index.shape[1])
    if key not in _CACHE:
        plan = _make_plan(edge_index, N_NODES)
        nc = _build_nc(plan)
        _CACHE[key] = (plan, nc)
    plan, nc = _CACHE[key]

    in_maps = _prepare_inputs(inputs, plan)
    if os.environ.get("MIXHOP_SIM"):
        from concourse import bass_interp

        sim = bass_interp.MultiCoreSim(nc, NC, num_workers=NC)
        for c in range(NC):
            for k, v in in_maps[c].items():
                sim.cores[c].tensor(k)[:] = v
        sim.simulate()
        outs = [{"out": np.array(sim.cores[c].mem_tensor("out"))} for c in range(NC)]
    else:
        res = bass_utils.run_bass_kernel_spmd(
            nc, in_maps, core_ids=list(range(NC)), trace=TRACE
        )
        LAST_EXEC_NS = res.exec_time_ns
        LAST_PROFILE = res.profile_json
        outs = res.results

    lpc, lpc_real = plan["lpc"], plan["lpc_real"]
    order = plan["order"]
    full = np.empty((N_NODES, N_CLS), dtype=np.float32)
    for c in range(NC):
        full[order[c::NC]] = outs[c]["out"][:lpc_real]
    return full


# revision 13
# speedup vs baseline: 1.3589x; 1.2515x over previous
"""MixHop GNN Bass kernel for 8 Trainium2 NeuronCores.

Self-contained: host-side preprocessing (numpy) + Bass/Tile device kernel.

Algorithm notes
---------------
Reference computes, per layer l (widths D_l = 128, 180, 180):
    P0 = X W0^T + b0
    P1 = A (X W1^T) + b1          (A = D^-1/2 (Adj + I) D^-1/2, GCN norm)
    P2 = A^2 (X W2^T) + b2
    X' = BN([P0 | P1 | P2])       (eval-mode affine)
then OUT = X4 linW^T + lin_b.

We fold BN into the weights/biases, and factorize A = S Adjhat S with
S = diag(dis), dis = 1/sqrt(deg).  So each hop is a *pure* gather +
segment-sum of unscaled rows plus cheap per-node scalings.

Device data flow per layer (per core; nodes sharded 6250/core):
    1. PE: Y = X_loc @ U   (U columns: [W1'(60)|pad|W2'(60)|pad|W0'(60)|pad], 192 wide)
    2. T12 = dis * Y[:, 0:128]; DMA -> stage; AllGather -> YT table [50176, 128]
    3. SpMM1: dma_gather YT rows per edge (dest-sorted blocks) -> DVE strided
       tensor_reduce per 128-dest block -> R [6250, 128]; R += T12 (self loop)
    4. Z2' = dis^2 * R[:, 64:124]; DMA; AllGather -> ZT [50176, 64]
    5. SpMM2: gather ZT, reduce -> R2 [6250, 64]; R2 += Z2' (self loop)
    6. X' cols: 0:60 = Y[:,128:188]+c0; 60:120 = dis*R[:,0:60]+c1;
       120:180 = dis*R2[:,0:60]+c2; PE-transpose -> X_T for next layer.

Edges are split in two passes by source table-row half (so dma_gather's
int16 indices stay < 32768), destination-sorted into 128-row blocks padded
to the per-block max in-degree (nodes are dealt to cores round-robin by
global degree rank, so block structure is identical across cores -> one
SPMD program).  Padding slots gather a guaranteed-zero row (pad rows have
dis = 0).
"""

import os
import numpy as np

# ---------------------------------------------------------------- problem dims
N_NODES = 50000
N_EDGES = 400000
D_IN = 128
H = 60
N_CLS = 40
EPS = 1e-5
NC = 8  # cores

CH_COLS = 24  # gather chunk: 24 block-columns = 3072 slots
STAGE = int(os.environ.get("MIXHOP_STAGE", "99"))

TRACE = bool(os.environ.get("MIXHOP_TRACE"))
LAST_EXEC_NS = None
LAST_PROFILE = None


# ================================================================ host planning
def _make_plan(edge_index, n_nodes):
    """Degree-based node permutation, per-core edge slot layout, chunk list."""
    row = np.asarray(edge_index[0], dtype=np.int64)
    col = np.asarray(edge_index[1], dtype=np.int64)
    E = row.shape[0]

    deg = np.bincount(col, minlength=n_nodes) + 1  # + self loop
    dis = (1.0 / np.sqrt(deg.astype(np.float64))).astype(np.float32)

    lpc_real = n_nodes // NC
    nblk = -(-lpc_real // 128)
    lpc = nblk * 128
    nrows = NC * lpc
    half = nrows // 2
    assert half <= 32768, (half, "int16 gather index range exceeded")

    # global rank by degree (desc).  Nodes are dealt to cores in windows of
    # NC*128 consecutive ranks (one 128-dest block per core per window), with
    # a greedy lo/hi half assignment that balances every destination's
    # source split (cuts the per-block max in-degree padding massively
    # compared with a random half split).
    order = np.argsort(-deg, kind="stable")  # perm: rank -> node
    rank = np.empty(n_nodes, dtype=np.int64)
    rank[order] = np.arange(n_nodes)

    # out-adjacency CSR over source nodes (edges sorted by row)
    o_src = np.argsort(row, kind="stable")
    dst_sorted = col[o_src]
    src_starts = np.searchsorted(row[o_src], np.arange(n_nodes + 1))

    rng_bal = np.random.default_rng(12345)
    bal = np.zeros(n_nodes, dtype=np.int64)  # cnt_lo - cnt_hi per dest
    colors = np.empty(n_nodes, dtype=np.int8)
    wsize = NC * 128
    half_cores = NC // 2
    for w0 in range(0, n_nodes, wsize):
        wnodes = order[w0 : min(w0 + wsize, n_nodes)]
        quota = len(wnodes) // 2  # lo slots in this window
        quota_hi = len(wnodes) - quota
        wperm = rng_bal.permutation(len(wnodes))
        for v in wnodes[wperm]:
            d = dst_sorted[src_starts[v] : src_starts[v + 1]]
            s = bal[d].sum()
            if quota == 0:
                c = 1
            elif quota_hi == 0:
                c = 0
            else:
                c = 0 if s <= 0 else 1
            colors[v] = c
            if c == 0:
                bal[d] += 1
                quota -= 1
            else:
                bal[d] -= 1
                quota_hi -= 1

    # deal within window: j-th lo node of window -> core j%4, local w*128+j//4
    core_of = np.empty(n_nodes, dtype=np.int64)
    local_of = np.empty(n_nodes, dtype=np.int64)
    for w0 in range(0, n_nodes, wsize):
        wnodes = order[w0 : min(w0 + wsize, n_nodes)]
        blk = w0 // wsize
        for base_core, grp in ((0, wnodes[colors[wnodes] == 0]),
                               (half_cores, wnodes[colors[wnodes] == 1])):
            j = np.arange(len(grp))
            core_of[grp] = base_core + j % half_cores
            local_of[grp] = blk * 128 + j // half_cores
    trow = core_of * lpc + local_of  # table row per node
    assert trow.max() < nrows
    # lo color <-> trow < half must agree
    assert (colors == (trow >= half)).all()

    # per-edge attributes
    e_core = core_of[col]
    e_local = local_of[col]
    e_src = trow[row]
    e_pass = (e_src >= half).astype(np.int64)

    # counts per (core, pass, local dest)
    cnt = np.zeros((NC, 2, lpc), dtype=np.int64)
    np.add.at(cnt, (e_core, e_pass, e_local), 1)

    # shared per-block K (max across cores and dests in block)
    kb = np.zeros((2, nblk), dtype=np.int64)
    for p in range(2):
        kb[p] = cnt[:, p, :].reshape(NC, nblk, 128).max(axis=(0, 2))

    cb = np.zeros((2, nblk), dtype=np.int64)  # column base per (pass, block)
    cb[0, 1:] = np.cumsum(kb[0])[:-1]
    cb[1, 1:] = np.cumsum(kb[1])[:-1]
    s_pass = [int(kb[0].sum() * 128), int(kb[1].sum() * 128)]
    base = [0, s_pass[0]]
    s_tot = s_pass[0] + s_pass[1]
    # round total slots to multiple of 16 columns is automatic (each block is
    # 128-slot aligned); s_tot % 16 == 0 trivially.

    # position of each edge within its (core, pass, local) group
    key = (e_core * 2 + e_pass) * lpc + e_local
    o = np.argsort(key, kind="stable")
    ks = key[o]
    grp_change = np.r_[0, np.flatnonzero(np.diff(ks)) + 1]
    grp_sizes = np.diff(np.r_[grp_change, E])
    pos_sorted = np.arange(E) - np.repeat(grp_change, grp_sizes)
    pos = np.empty(E, dtype=np.int64)
    pos[o] = pos_sorted

    # slot index (within the core's idx array) for each edge
    e_blk = e_local // 128
    e_p128 = e_local % 128
    e_col = cb[e_pass, e_blk] + pos
    e_slot = np.array(base)[e_pass] + e_col * 128 + e_p128

    # local gather index for each edge (within its half)
    e_idx = (e_src - np.where(e_pass == 1, half, 0)).astype(np.int16)

    # dummy (padding) targets: a pad row in each half (zero-staged since dis=0)
    assert lpc_real < lpc, "need at least one pad row per core for dummies"
    dummy_lo = np.int16(lpc_real)  # core 0 pad row, trow < half
    dummy_hi = np.int16((NC // 2) * lpc + lpc_real - half)

    # per-core idx arrays, [128, s_tot//16] int16 (16-part wrap, replicated x8)
    idx_all = np.empty((NC, s_tot), dtype=np.int16)
    idx_all[:, : s_pass[0]] = dummy_lo
    idx_all[:, s_pass[0] :] = dummy_hi
    idx_all[e_core, e_slot] = e_idx
    idx_wrapped = np.empty((NC, 16, s_tot // 16), dtype=np.int16)
    for c in range(NC):
        idx_wrapped[c] = idx_all[c].reshape(-1, 16).T
    idx_in = np.ascontiguousarray(np.tile(idx_wrapped, (1, 8, 1)))

    # chunks: walk (pass, block) cols, split into <= CH_COLS col chunks.
    # chunk = dict(pass, col_start, ncols, pieces=[(blk, coff, w, first)])
    chunks = []
    for p in range(2):
        cur = dict(p=p, col0=int(cb[p, 0]), ncols=0, pieces=[])
        for b in range(nblk):
            rem = int(kb[p, b])
            first = True
            while rem > 0:
                if cur["ncols"] == CH_COLS:
                    chunks.append(cur)
                    cur = dict(
                        p=p, col0=cur["col0"] + cur["ncols"], ncols=0, pieces=[]
                    )
                take = min(rem, CH_COLS - cur["ncols"])
                cur["pieces"].append((b, cur["ncols"], take, first))
                cur["ncols"] += take
                rem -= take
                first = False
        if cur["ncols"]:
            chunks.append(cur)

    # blocks never touched by a reduce-write in a layer (need per-layer zeroing)
    zero_blocks = [b for b in range(nblk) if kb[0, b] == 0 and kb[1, b] == 0]
    lo_blocks = set(b for b in range(nblk) if kb[0, b] > 0)
    # blocks whose first write happens in pass hi
    hi_first_blocks = set(
        b for b in range(nblk) if kb[0, b] == 0 and kb[1, b] > 0
    )

    # node ids per (core, local)
    nodes_by_core = np.empty((NC, lpc_real), dtype=np.int64)
    nodes_by_core[core_of, local_of] = np.arange(n_nodes)

    # per-core dis tiles [128, nblk]: partition p, block b -> local b*128+p
    disb = np.zeros((NC, 128, nblk), dtype=np.float32)
    for c in range(NC):
        d = np.zeros(lpc, dtype=np.float32)
        d[:lpc_real] = dis[nodes_by_core[c]]
        disb[c] = d.reshape(nblk, 128).T

    return dict(
        E=E,
        lpc_real=lpc_real,
        nblk=nblk,
        lpc=lpc,
        nrows=nrows,
        half=half,
        nodes_by_core=nodes_by_core,
        idx_in=idx_in,
        chunks=chunks,
        zero_blocks=zero_blocks,
        hi_first_blocks=hi_first_blocks,
        s_tot=s_tot,
        disb=disb,
        kb=kb,
    )


def _fold_weights(inp):
    """Fold BN into conv weights/biases. Returns per-layer U [Din,192], biases."""
    f = np.float32
    Us, biases = [], []
    for li, (wname, bname, bn) in enumerate(
        [("conv1", None, "bn1"), ("conv2", None, "bn2"), ("conv3", None, "bn3")]
    ):
        W = np.asarray(inp[wname + "_W"], dtype=f)  # [3, 60, Din]
        b = np.asarray(inp[wname + "_b"], dtype=f)  # [3, 60]
        g = np.asarray(inp[bn + "_g"], dtype=f)
        bb = np.asarray(inp[bn + "_b"], dtype=f)
        m = np.asarray(inp[bn + "_m"], dtype=f)
        v = np.asarray(inp[bn + "_v"], dtype=f)
        s = g / np.sqrt(v + EPS)
        t = bb - m * s
        s3 = s.reshape(3, H)
        t3 = t.reshape(3, H)
        Din = W.shape[2]
        U = np.zeros((Din, 192), dtype=f)
        U[:, 0:H] = (W[1] * s3[1][:, None]).T
        U[:, 64 : 64 + H] = (W[2] * s3[2][:, None]).T
        U[:, 128 : 128 + H] = (W[0] * s3[0][:, None]).T
        c0 = b[0] * s3[0] + t3[0]
        c1 = b[1] * s3[1] + t3[1]
        c2 = b[2] * s3[2] + t3[2]
        Us.append(U)
        biases.append((c0, c1, c2))
    return Us, biases


# ================================================================ device build
def _build_nc(plan):
    import concourse.bass as bass
    import concourse.bacc as bacc
    import concourse.mybir as mybir
    import concourse.tile as tile
    from concourse.masks import make_identity

    f32 = mybir.dt.float32
    i16 = mybir.dt.int16
    nblk = plan["nblk"]
    lpc = plan["lpc"]
    nrows = plan["nrows"]
    half = plan["half"]
    s_tot = plan["s_tot"]
    chunks = plan["chunks"]
    AX = mybir.AxisListType.X
    ADD = mybir.AluOpType.add
    MUL = mybir.AluOpType.mult

    nc = bacc.Bacc("TRN2", target_bir_lowering=False, debug=False, num_devices=NC)

    # ---- I/O
    xT = nc.declare_dram_parameter("xT", [128, lpc], f32, isOutput=False)
    idx_d = nc.declare_dram_parameter("idx", [128, s_tot // 16], i16, isOutput=False)
    disb_d = nc.declare_dram_parameter("disb", [128, nblk], f32, isOutput=False)
    dis2b_d = nc.declare_dram_parameter("dis2b", [128, nblk], f32, isOutput=False)
    u_d = [
        nc.declare_dram_parameter("u0", [128, 192], f32, isOutput=False),
        (
            nc.declare_dram_parameter("u1a", [128, 192], f32, isOutput=False),
            nc.declare_dram_parameter("u1b", [64, 192], f32, isOutput=False),
        ),
        (
            nc.declare_dram_parameter("u2a", [128, 192], f32, isOutput=False),
            nc.declare_dram_parameter("u2b", [64, 192], f32, isOutput=False),
        ),
    ]
    bias_d = [
        [
            nc.declare_dram_parameter(f"bias{l}_{k}", [128, 64], f32, isOutput=False)
            for k in range(3)
        ]
        for l in range(3)
    ]
    lwa_d = nc.declare_dram_parameter("lwa", [128, N_CLS], f32, isOutput=False)
    lwb_d = nc.declare_dram_parameter("lwb", [64, N_CLS], f32, isOutput=False)
    blin_d = nc.declare_dram_parameter("blin", [128, N_CLS], f32, isOutput=False)
    out_d = nc.declare_dram_parameter("out", [lpc, N_CLS], f32, isOutput=True)

    # ---- internal DRAM (per layer to keep collective buffers single-writer)
    ystage = [nc.dram_tensor(f"ystage{l}", [lpc, 128], f32) for l in range(3)]
    yt = [
        nc.dram_tensor(f"yt{l}", [nrows, 128], f32, addr_space="Shared")
        for l in range(3)
    ]
    zstage = [nc.dram_tensor(f"zstage{l}", [lpc, 64], f32) for l in range(3)]
    zt = [
        nc.dram_tensor(f"zt{l}", [nrows, 64], f32, addr_space="Shared")
        for l in range(3)
    ]
    rg = [list(range(NC))]

    with tile.TileContext(nc) as tc, \
            tc.tile_pool(name="const", bufs=1) as const, \
            tc.tile_pool(name="big", bufs=1) as big:
        # persistent SBUF state
        XTa = big.tile([128, lpc], f32, tag="XTa")
        XTb = big.tile([64, lpc], f32, tag="XTb")
        T12 = big.tile([128, nblk * 128], f32, tag="T12")
        Y0 = big.tile([128, nblk * 64], f32, tag="Y0")
        R1 = big.tile([128, nblk * 128], f32, tag="R1")
        R2 = big.tile([128, nblk * 64], f32, tag="R2")
        ZL = big.tile([128, nblk * 64], f32, tag="ZL")
        OUTALL = big.tile([128, nblk * N_CLS], f32, tag="OUTALL")
        idx_sb = const.tile([128, s_tot // 16], i16)
        disb_sb = const.tile([128, nblk], f32)
        dis2b_sb = const.tile([128, nblk], f32)
        ident = const.tile([128, 128], f32)
        make_identity(nc, ident)

        nc.sync.dma_start(out=idx_sb[:], in_=idx_d[:])
        nc.sync.dma_start(out=disb_sb[:], in_=disb_d[:])
        nc.sync.dma_start(out=dis2b_sb[:], in_=dis2b_d[:])
        nc.sync.dma_start(out=XTa[:], in_=xT[:])

        u_sb = []
        u0 = const.tile([128, 192], f32)
        nc.sync.dma_start(out=u0[:], in_=u_d[0][:])
        u_sb.append((u0, None))
        for l in (1, 2):
            ua = const.tile([128, 192], f32, tag=f"u{l}a")
            ub = const.tile([64, 192], f32, tag=f"u{l}b")
            nc.sync.dma_start(out=ua[:], in_=u_d[l][0][:])
            nc.sync.dma_start(out=ub[:], in_=u_d[l][1][:])
            u_sb.append((ua, ub))
        bias_sb = []
        for l in range(3):
            row = []
            for k in range(3):
                t = const.tile([128, 64], f32, tag=f"b{l}{k}")
                nc.sync.dma_start(out=t[:], in_=bias_d[l][k][:])
                row.append(t)
            bias_sb.append(row)
        lwa = const.tile([128, N_CLS], f32)
        lwb = const.tile([64, N_CLS], f32)
        blin = const.tile([128, N_CLS], f32)
        nc.sync.dma_start(out=lwa[:], in_=lwa_d[:])
        nc.sync.dma_start(out=lwb[:], in_=lwb_d[:])
        nc.sync.dma_start(out=blin[:], in_=blin_d[:])

        # zero-init accumulators (pad-only blocks rely on this)
        nc.any.memset(R1[:], 0.0)
        nc.any.memset(R2[:], 0.0)
        nc.any.memset(ZL[:], 0.0)

        with (
            tc.tile_pool(name="psum", bufs=2, space="PSUM") as psum,
            tc.tile_pool(name="tpsum", bufs=2, space="PSUM") as tpsum,
            tc.tile_pool(name="g1p", bufs=2) as g1p,
            tc.tile_pool(name="g2p", bufs=2) as g2p,
            tc.tile_pool(name="work", bufs=4) as work,
        ):
            for l in range(3):
                ua, ub = u_sb[l]
                # ---------------- A: dense Y = X @ U ; split T12 / Y0
                for rb in range(nblk):
                    ps = psum.tile([128, 192], f32, tag="ps")
                    sl = slice(rb * 128, (rb + 1) * 128)
                    use_b = ub is not None and STAGE >= 6
                    nc.tensor.matmul(
                        out=ps[:],
                        lhsT=XTa[:, sl],
                        rhs=ua[:],
                        start=True,
                        stop=not use_b,
                    )
                    if use_b:
                        nc.tensor.matmul(
                            out=ps[:], lhsT=XTb[:, sl], rhs=ub[:],
                            start=False, stop=True,
                        )
                    nc.any.tensor_copy(out=T12[:, sl], in_=ps[:, 0:128])
                    nc.any.tensor_copy(
                        out=Y0[:, rb * 64 : (rb + 1) * 64], in_=ps[:, 128:192]
                    )
                # T12 *= dis (per row)  [128, nblk, 128] * [128, nblk, 1]
                nc.vector.tensor_tensor(
                    out=T12[:].rearrange("p (b e) -> p b e", e=128),
                    in0=T12[:].rearrange("p (b e) -> p b e", e=128),
                    in1=disb_sb[:].unsqueeze(2).to_broadcast([128, nblk, 128]),
                    op=MUL,
                )
                if STAGE < 2:
                    continue
                nc.sync.dma_start(
                    out=ystage[l][:].rearrange("(b p) e -> p b e", p=128),
                    in_=T12[:].rearrange("p (b e) -> p b e", e=128),
                )
                nc.gpsimd.collective_compute(
                    "AllGather", mybir.AluOpType.bypass,
                    replica_groups=rg, ins=[ystage[l][:]], outs=[yt[l][:]],
                )
                # per-layer zero of never-written blocks
                for b in plan["zero_blocks"]:
                    nc.any.memset(R1[:, b * 128 : (b + 1) * 128], 0.0)
                    nc.any.memset(R2[:, b * 64 : (b + 1) * 64], 0.0)

                # ---------------- B: SpMM1 (gather YT, reduce per block)
                if STAGE < 3:
                    continue
                for ch in chunks:
                    ncols = ch["ncols"]
                    slot0 = (0 if ch["p"] == 0 else plan["kb"][0].sum() * 128) + ch[
                        "col0"
                    ] * 128
                    gt = g1p.tile([128, CH_COLS, 128], f32, tag="g1")
                    src = yt[l][0:half, :] if ch["p"] == 0 else yt[l][half:nrows, :]
                    nc.gpsimd.dma_gather(
                        out_ap=gt[:, :ncols, :],
                        in_ap=src,
                        idxs_ap=idx_sb[:, slot0 // 16 : (slot0 + ncols * 128) // 16],
                        num_idxs=ncols * 128,
                        num_idxs_reg=ncols * 128,
                        elem_size=128,
                        single_packet=False,
                    )
                    for (b, coff, w, first) in ch["pieces"]:
                        first_write = first and (
                            ch["p"] == 0 or b in plan["hi_first_blocks"]
                        )
                        view = gt[:, coff : coff + w, :].rearrange("p c e -> p e c")
                        bsl = slice(b * 128, (b + 1) * 128)
                        if first_write:
                            nc.vector.tensor_reduce(
                                out=R1[:, bsl], in_=view, axis=AX, op=ADD
                            )
                        else:
                            tmp = work.tile([128, 128], f32, tag="t1")
                            nc.vector.tensor_reduce(
                                out=tmp[:], in_=view, axis=AX, op=ADD
                            )
                            nc.any.tensor_tensor(
                                out=R1[:, bsl], in0=R1[:, bsl], in1=tmp[:], op=ADD
                            )
                # self loop: R1 += T12
                if STAGE < 4:
                    continue
                nc.vector.tensor_tensor(out=R1[:], in0=R1[:], in1=T12[:], op=ADD)
                # Z2' = dis^2 * R1[:, 64:124] -> ZL (cols 0:60 of each 64-block)
                r1v = R1[:].rearrange("p (b e) -> p b e", e=128)
                zlv = ZL[:].rearrange("p (b e) -> p b e", e=64)
                nc.vector.tensor_tensor(
                    out=zlv[:, :, 0:60],
                    in0=r1v[:, :, 64:124],
                    in1=dis2b_sb[:].unsqueeze(2).to_broadcast([128, nblk, 60]),
                    op=MUL,
                )
                nc.sync.dma_start(
                    out=zstage[l][:].rearrange("(b p) e -> p b e", p=128),
                    in_=zlv,
                )
                nc.gpsimd.collective_compute(
                    "AllGather", mybir.AluOpType.bypass,
                    replica_groups=rg, ins=[zstage[l][:]], outs=[zt[l][:]],
                )

                # ---------------- C: SpMM2 (gather ZT, reduce)
                if STAGE < 5:
                    continue
                for ch in chunks:
                    ncols = ch["ncols"]
                    slot0 = (0 if ch["p"] == 0 else plan["kb"][0].sum() * 128) + ch[
                        "col0"
                    ] * 128
                    gt = g2p.tile([128, CH_COLS, 64], f32, tag="g2")
                    src = zt[l][0:half, :] if ch["p"] == 0 else zt[l][half:nrows, :]
                    nc.gpsimd.dma_gather(
                        out_ap=gt[:, :ncols, :],
                        in_ap=src,
                        idxs_ap=idx_sb[:, slot0 // 16 : (slot0 + ncols * 128) // 16],
                        num_idxs=ncols * 128,
                        num_idxs_reg=ncols * 128,
                        elem_size=64,
                        single_packet=False,
                    )
                    for (b, coff, w, first) in ch["pieces"]:
                        first_write = first and (
                            ch["p"] == 0 or b in plan["hi_first_blocks"]
                        )
                        view = gt[:, coff : coff + w, :].rearrange("p c e -> p e c")
                        bsl = slice(b * 64, (b + 1) * 64)
                        if first_write:
                            nc.vector.tensor_reduce(
                                out=R2[:, bsl], in_=view, axis=AX, op=ADD
                            )
                        else:
                            tmp = work.tile([128, 64], f32, tag="t2")
                            nc.vector.tensor_reduce(
                                out=tmp[:], in_=view, axis=AX, op=ADD
                            )
                            nc.any.tensor_tensor(
                                out=R2[:, bsl], in0=R2[:, bsl], in1=tmp[:], op=ADD
                            )
                # self loop: R2 += ZL
                nc.vector.tensor_tensor(out=R2[:], in0=R2[:], in1=ZL[:], op=ADD)

                # ---------------- D: assemble X' and transpose to X_T
                if STAGE < 6:
                    continue
                b0, b1, b2 = bias_sb[l]
                r2v = R2[:].rearrange("p (b e) -> p b e", e=64)
                y0v = Y0[:].rearrange("p (b e) -> p b e", e=64)
                for rb in range(nblk):
                    xn = work.tile([128, 192], f32, tag="xn")
                    nc.any.memset(xn[:, 180:192], 0.0)
                    # P0
                    nc.any.tensor_tensor(
                        out=xn[:, 0:60], in0=y0v[:, rb, 0:60],
                        in1=b0[:, 0:60], op=ADD,
                    )
                    # P1 = dis * R1 + c1
                    nc.any.tensor_scalar(
                        out=xn[:, 60:120],
                        in0=r1v[:, rb, 0:60],
                        scalar1=disb_sb[:, rb : rb + 1],
                        scalar2=None,
                        op0=MUL,
                    )
                    nc.any.tensor_tensor(
                        out=xn[:, 60:120], in0=xn[:, 60:120],
                        in1=b1[:, 0:60], op=ADD,
                    )
                    # P2 = dis * R2 + c2
                    nc.any.tensor_scalar(
                        out=xn[:, 120:180],
                        in0=r2v[:, rb, 0:60],
                        scalar1=disb_sb[:, rb : rb + 1],
                        scalar2=None,
                        op0=MUL,
                    )
                    nc.any.tensor_tensor(
                        out=xn[:, 120:180], in0=xn[:, 120:180],
                        in1=b2[:, 0:60], op=ADD,
                    )
                    sl = slice(rb * 128, (rb + 1) * 128)
                    pt = tpsum.tile([128, 128], f32, tag="pt")
                    nc.tensor.transpose(out=pt[:], in_=xn[:, 0:128], identity=ident[:])
                    nc.any.tensor_copy(out=XTa[:, sl], in_=pt[:])
                    pt2 = tpsum.tile([64, 128], f32, tag="pt2")
                    nc.tensor.transpose(
                        out=pt2[:], in_=xn[:, 128:192], identity=ident[:]
                    )
                    nc.any.tensor_copy(out=XTb[:, sl], in_=pt2[:])

            # ---------------- final linear
            for rb in range(nblk):
                sl = slice(rb * 128, (rb + 1) * 128)
                ps = psum.tile([128, N_CLS], f32, tag="pf")
                nc.tensor.matmul(
                    out=ps[:], lhsT=XTa[:, sl], rhs=lwa[:], start=True,
                    stop=STAGE < 6,
                )
                if STAGE >= 6:
                    nc.tensor.matmul(
                        out=ps[:], lhsT=XTb[:, sl], rhs=lwb[:], start=False, stop=True
                    )
                nc.any.tensor_tensor(
                    out=OUTALL[:, rb * N_CLS : (rb + 1) * N_CLS],
                    in0=ps[:], in1=blin[:], op=ADD,
                )
            nc.sync.dma_start(
                out=out_d[:].rearrange("(b p) c -> p b c", p=128),
                in_=OUTALL[:].rearrange("p (b c) -> p b c", c=N_CLS),
            )

    nc.compile()
    return nc


# ================================================================ entry point
def _prepare_inputs(inputs, plan):
    """Build per-core in_maps."""
    f = np.float32
    x = np.asarray(inputs["x"], dtype=f)
    nbc = plan["nodes_by_core"]
    lpc, lpc_real, nblk = plan["lpc"], plan["lpc_real"], plan["nblk"]
    Us, biases = _fold_weights(inputs)
    lin_W = np.asarray(inputs["lin_W"], dtype=f)  # [40, 180]
    lin_b = np.asarray(inputs["lin_b"], dtype=f)
    lwT = np.zeros((192, N_CLS), dtype=f)
    lwT[0:180, :] = lin_W.T
    blin = np.tile(lin_b[None, :], (128, 1)).astype(f)

    def repl_bias(c):
        t = np.zeros((128, 64), dtype=f)
        t[:, 0:60] = c[None, :]
        return t

    in_maps = []
    for c in range(NC):
        m = {}
        xs = np.zeros((lpc, D_IN), dtype=f)
        xs[:lpc_real] = x[nbc[c]]
        m["xT"] = np.ascontiguousarray(xs.T)
        m["idx"] = plan["idx_in"][c]
        m["disb"] = np.ascontiguousarray(plan["disb"][c])
        m["dis2b"] = np.ascontiguousarray(plan["disb"][c] ** 2)
        m["u0"] = Us[0]
        m["u1a"] = np.ascontiguousarray(Us[1][0:128])
        u1b = np.zeros((64, 192), dtype=f)
        u1b[0:52] = Us[1][128:180]
        m["u1b"] = u1b
        m["u2a"] = np.ascontiguousarray(Us[2][0:128])
        u2b = np.zeros((64, 192), dtype=f)
        u2b[0:52] = Us[2][128:180]
        m["u2b"] = u2b
        for l in range(3):
            for k in range(3):
                m[f"bias{l}_{k}"] = repl_bias(biases[l][k])
        m["lwa"] = np.ascontiguousarray(lwT[0:128])
        m["lwb"] = np.ascontiguousarray(lwT[128:192])
        m["blin"] = blin
        in_maps.append(m)
    return in_maps


_CACHE = {}


def _install_ntff_shim():
    """Provide antenv.axon_hooks (missing in this image) so trace=True works."""
    import sys, types, ctypes, contextlib

    try:
        from antenv.axon_hooks import get_axon_ntff_profile_hook  # noqa: F401

        return
    except ImportError:
        pass
    so_path = "/opt/axon/libaxon_pjrt.so"
    hook = None
    try:
        lib = ctypes.CDLL(so_path)
        if hasattr(lib, "axon_start_nrt_profile"):
            lib.axon_start_nrt_profile.argtypes = [
                ctypes.POINTER(ctypes.c_int64),
                ctypes.c_size_t,
            ]
            lib.axon_start_nrt_profile.restype = ctypes.c_int64
            lib.axon_stop_nrt_profile.argtypes = [ctypes.c_char_p]
            lib.axon_stop_nrt_profile.restype = ctypes.c_int64

            @contextlib.contextmanager
            def hook(output_dir, device_ids):
                import jax

                jax.devices()
                if device_ids:
                    ids = (ctypes.c_int64 * len(device_ids))(*device_ids)
                    rc = lib.axon_start_nrt_profile(ids, len(device_ids))
                else:
                    rc = lib.axon_start_nrt_profile(None, 0)
                if rc != 0:
                    raise RuntimeError(f"axon_start_nrt_profile rc={rc}")
                try:
                    yield
                finally:
                    n = lib.axon_stop_nrt_profile(str(output_dir).encode())
                    print(f"profile: {n} file(s) written to {output_dir}")

    except OSError:
        pass
    mod = types.ModuleType("antenv.axon_hooks")
    mod.get_axon_ntff_profile_hook = lambda: hook
    mod.set_axon_ntff_profile_hook = lambda h: None
    sys.modules["antenv.axon_hooks"] = mod


def kernel(**inputs):
    global LAST_EXEC_NS, LAST_PROFILE
    from concourse import bass_utils

    if TRACE:
        _install_ntff_shim()
        bass_utils.upload_artifacts = lambda tmpdir: tmpdir

    edge_index = np.asarray(inputs["edge_index"])
    key = ("plan", edge_index.shape[1])
    if key not in _CACHE:
        plan = _make_plan(edge_index, N_NODES)
        nc = _build_nc(plan)
        _CACHE[key] = (plan, nc)
    plan, nc = _CACHE[key]

    in_maps = _prepare_inputs(inputs, plan)
    if os.environ.get("MIXHOP_SIM"):
        from concourse import bass_interp

        sim = bass_interp.MultiCoreSim(nc, NC, num_workers=NC)
        for c in range(NC):
            for k, v in in_maps[c].items():
                sim.cores[c].tensor(k)[:] = v
        sim.simulate()
        outs = [{"out": np.array(sim.cores[c].mem_tensor("out"))} for c in range(NC)]
    else:
        res = bass_utils.run_bass_kernel_spmd(
            nc, in_maps, core_ids=list(range(NC)), trace=TRACE
        )
        LAST_EXEC_NS = res.exec_time_ns
        LAST_PROFILE = res.profile_json
        outs = res.results

    lpc, lpc_real = plan["lpc"], plan["lpc_real"]
    nbc = plan["nodes_by_core"]
    full = np.empty((N_NODES, N_CLS), dtype=np.float32)
    for c in range(NC):
        full[nbc[c]] = outs[c]["out"][:lpc_real]
    return full


# revision 14
# speedup vs baseline: 1.8306x; 1.3471x over previous
"""MixHop GNN Bass kernel for 8 Trainium2 NeuronCores.

Self-contained: host-side preprocessing (numpy) + Bass/Tile device kernel.

Algorithm notes
---------------
Reference computes, per layer l (widths D_l = 128, 180, 180):
    P0 = X W0^T + b0
    P1 = A (X W1^T) + b1          (A = D^-1/2 (Adj + I) D^-1/2, GCN norm)
    P2 = A^2 (X W2^T) + b2
    X' = BN([P0 | P1 | P2])       (eval-mode affine)
then OUT = X4 linW^T + lin_b.

We fold BN into the weights/biases, and factorize A = S Adjhat S with
S = diag(dis), dis = 1/sqrt(deg).  So each hop is a *pure* gather +
segment-sum of unscaled rows plus cheap per-node scalings.

Device data flow per layer (per core; nodes sharded 6250/core):
    1. PE: Y = X_loc @ U   (U columns: [W1'(60)|pad|W2'(60)|pad|W0'(60)|pad], 192 wide)
    2. T12 = dis * Y[:, 0:128]; DMA -> stage; AllGather -> YT table [50176, 128]
    3. SpMM1: dma_gather YT rows per edge (dest-sorted blocks) -> DVE strided
       tensor_reduce per 128-dest block -> R [6250, 128]; R += T12 (self loop)
    4. Z2' = dis^2 * R[:, 64:124]; DMA; AllGather -> ZT [50176, 64]
    5. SpMM2: gather ZT, reduce -> R2 [6250, 64]; R2 += Z2' (self loop)
    6. X' cols: 0:60 = Y[:,128:188]+c0; 60:120 = dis*R[:,0:60]+c1;
       120:180 = dis*R2[:,0:60]+c2; PE-transpose -> X_T for next layer.

Edges are split in two passes by source table-row half (so dma_gather's
int16 indices stay < 32768), destination-sorted into 128-row blocks padded
to the per-block max in-degree (nodes are dealt to cores round-robin by
global degree rank, so block structure is identical across cores -> one
SPMD program).  Padding slots gather a guaranteed-zero row (pad rows have
dis = 0).
"""

import os
import numpy as np

# ---------------------------------------------------------------- problem dims
N_NODES = 50000
N_EDGES = 400000
D_IN = 128
H = 60
N_CLS = 40
EPS = 1e-5
NC = 8  # cores

CH_COLS = 8  # gather chunk: 8 block-columns = 1024 slots (= one SWDGE ring)
STAGE = int(os.environ.get("MIXHOP_STAGE", "99"))

TRACE = bool(os.environ.get("MIXHOP_TRACE"))
LAST_EXEC_NS = None
LAST_PROFILE = None


# ================================================================ host planning
def _make_plan(edge_index, n_nodes):
    """Degree-based node permutation, per-core edge slot layout, chunk list."""
    row = np.asarray(edge_index[0], dtype=np.int64)
    col = np.asarray(edge_index[1], dtype=np.int64)
    E = row.shape[0]

    deg = np.bincount(col, minlength=n_nodes) + 1  # + self loop
    dis = (1.0 / np.sqrt(deg.astype(np.float64))).astype(np.float32)

    lpc_real = n_nodes // NC
    nblk = -(-lpc_real // 128)
    lpc = nblk * 128
    nrows = NC * lpc
    half = nrows // 2
    assert half <= 32768, (half, "int16 gather index range exceeded")

    # global rank by degree (desc).  Nodes are dealt to cores in windows of
    # NC*128 consecutive ranks (one 128-dest block per core per window), with
    # a greedy lo/hi half assignment that balances every destination's
    # source split (cuts the per-block max in-degree padding massively
    # compared with a random half split).
    order = np.argsort(-deg, kind="stable")  # perm: rank -> node
    rank = np.empty(n_nodes, dtype=np.int64)
    rank[order] = np.arange(n_nodes)

    # out-adjacency CSR over source nodes (edges sorted by row)
    o_src = np.argsort(row, kind="stable")
    dst_sorted = col[o_src]
    src_starts = np.searchsorted(row[o_src], np.arange(n_nodes + 1))

    rng_bal = np.random.default_rng(12345)
    bal = np.zeros(n_nodes, dtype=np.int64)  # cnt_lo - cnt_hi per dest
    colors = np.zeros(n_nodes, dtype=np.int8)
    half_cores = NC // 2
    quota = [n_nodes // 2, n_nodes - n_nodes // 2]
    for v in rng_bal.permutation(n_nodes):
        d = dst_sorted[src_starts[v] : src_starts[v + 1]]
        if quota[0] == 0:
            c = 1
        elif quota[1] == 0:
            c = 0
        else:
            c = 0 if bal[d].sum() <= 0 else 1
        colors[v] = c
        bal[d] += 1 - 2 * c
        quota[c] -= 1
    # refinement: coordinate descent on sum 2^|bal| (exp potential -> min max)
    net = 0
    for _ in range(4):
        for v in rng_bal.permutation(n_nodes):
            d = dst_sorted[src_starts[v] : src_starts[v + 1]]
            if len(d) == 0:
                continue
            s = 1 - 2 * colors[v]
            b0 = bal[d]
            dcost = (2.0 ** np.abs(b0 - 2 * s) - 2.0 ** np.abs(b0)).sum()
            if dcost < 0 and abs(net - 2 * s) <= 16:
                colors[v] = 1 - colors[v]
                bal[d] = b0 - 2 * s
                net -= 2 * s
    while net != 0:
        surplus = 0 if net > 0 else 1
        cand = np.flatnonzero(colors == surplus)
        cand = rng_bal.choice(cand, size=min(2000, len(cand)), replace=False)
        best, bestc = None, None
        for v in cand:
            d = dst_sorted[src_starts[v] : src_starts[v + 1]]
            s = 1 - 2 * colors[v]
            dc = (2.0 ** np.abs(bal[d] - 2 * s) - 2.0 ** np.abs(bal[d])).sum()
            if bestc is None or dc < bestc:
                best, bestc = v, dc
        v = best
        d = dst_sorted[src_starts[v] : src_starts[v + 1]]
        s = 1 - 2 * colors[v]
        colors[v] = 1 - colors[v]
        bal[d] -= 2 * s
        net -= 2 * s

    # deal: j-th node (degree-rank order) of color c -> core j%4, local j//4
    core_of = np.empty(n_nodes, dtype=np.int64)
    local_of = np.empty(n_nodes, dtype=np.int64)
    for base_core, cv in ((0, 0), (half_cores, 1)):
        grp = order[colors[order] == cv]
        j = np.arange(len(grp))
        core_of[grp] = base_core + j % half_cores
        local_of[grp] = j // half_cores
    assert local_of.max() < lpc_real + 1
    trow = core_of * lpc + local_of  # table row per node
    assert trow.max() < nrows
    assert (colors == (trow >= half)).all()

    # per-edge attributes
    e_core = core_of[col]
    e_local = local_of[col]
    e_src = trow[row]
    e_pass = (e_src >= half).astype(np.int64)

    # counts per (core, pass, local dest)
    cnt = np.zeros((NC, 2, lpc), dtype=np.int64)
    np.add.at(cnt, (e_core, e_pass, e_local), 1)

    # shared per-block K (max across cores and dests in block)
    kb = np.zeros((2, nblk), dtype=np.int64)
    for p in range(2):
        kb[p] = cnt[:, p, :].reshape(NC, nblk, 128).max(axis=(0, 2))

    cb = np.zeros((2, nblk), dtype=np.int64)  # column base per (pass, block)
    cb[0, 1:] = np.cumsum(kb[0])[:-1]
    cb[1, 1:] = np.cumsum(kb[1])[:-1]
    s_pass = [int(kb[0].sum() * 128), int(kb[1].sum() * 128)]
    base = [0, s_pass[0]]
    s_tot = s_pass[0] + s_pass[1]
    # round total slots to multiple of 16 columns is automatic (each block is
    # 128-slot aligned); s_tot % 16 == 0 trivially.

    # position of each edge within its (core, pass, local) group
    key = (e_core * 2 + e_pass) * lpc + e_local
    o = np.argsort(key, kind="stable")
    ks = key[o]
    grp_change = np.r_[0, np.flatnonzero(np.diff(ks)) + 1]
    grp_sizes = np.diff(np.r_[grp_change, E])
    pos_sorted = np.arange(E) - np.repeat(grp_change, grp_sizes)
    pos = np.empty(E, dtype=np.int64)
    pos[o] = pos_sorted

    # slot index (within the core's idx array) for each edge
    e_blk = e_local // 128
    e_p128 = e_local % 128
    e_col = cb[e_pass, e_blk] + pos
    e_slot = np.array(base)[e_pass] + e_col * 128 + e_p128

    # local gather index for each edge (within its half)
    e_idx = (e_src - np.where(e_pass == 1, half, 0)).astype(np.int16)

    # dummy (padding) targets: a pad row in each half (zero-staged since dis=0)
    assert lpc_real < lpc, "need at least one pad row per core for dummies"
    dummy_lo = np.int16(lpc_real)  # core 0 pad row, trow < half
    dummy_hi = np.int16((NC // 2) * lpc + lpc_real - half)

    # per-core idx arrays, [128, s_tot//16] int16 (16-part wrap, replicated x8)
    idx_all = np.empty((NC, s_tot), dtype=np.int16)
    idx_all[:, : s_pass[0]] = dummy_lo
    idx_all[:, s_pass[0] :] = dummy_hi
    idx_all[e_core, e_slot] = e_idx
    idx_wrapped = np.empty((NC, 16, s_tot // 16), dtype=np.int16)
    for c in range(NC):
        idx_wrapped[c] = idx_all[c].reshape(-1, 16).T
    idx_in = np.ascontiguousarray(np.tile(idx_wrapped, (1, 8, 1)))

    # chunks: walk (pass, block) cols, split into <= CH_COLS col chunks.
    # chunk = dict(pass, col_start, ncols, pieces=[(blk, coff, w, first)])
    chunks = []
    for p in range(2):
        cur = dict(p=p, col0=int(cb[p, 0]), ncols=0, pieces=[])
        for b in range(nblk):
            rem = int(kb[p, b])
            first = True
            while rem > 0:
                if cur["ncols"] == CH_COLS:
                    chunks.append(cur)
                    cur = dict(
                        p=p, col0=cur["col0"] + cur["ncols"], ncols=0, pieces=[]
                    )
                take = min(rem, CH_COLS - cur["ncols"])
                cur["pieces"].append((b, cur["ncols"], take, first))
                cur["ncols"] += take
                rem -= take
                first = False
        if cur["ncols"]:
            chunks.append(cur)

    # blocks never touched by a reduce-write in a layer (need per-layer zeroing)
    zero_blocks = [b for b in range(nblk) if kb[0, b] == 0 and kb[1, b] == 0]
    lo_blocks = set(b for b in range(nblk) if kb[0, b] > 0)
    # blocks whose first write happens in pass hi
    hi_first_blocks = set(
        b for b in range(nblk) if kb[0, b] == 0 and kb[1, b] > 0
    )

    # node ids per (core, local)
    nodes_by_core = np.empty((NC, lpc_real), dtype=np.int64)
    nodes_by_core[core_of, local_of] = np.arange(n_nodes)

    # per-core dis tiles [128, nblk]: partition p, block b -> local b*128+p
    disb = np.zeros((NC, 128, nblk), dtype=np.float32)
    for c in range(NC):
        d = np.zeros(lpc, dtype=np.float32)
        d[:lpc_real] = dis[nodes_by_core[c]]
        disb[c] = d.reshape(nblk, 128).T

    return dict(
        E=E,
        lpc_real=lpc_real,
        nblk=nblk,
        lpc=lpc,
        nrows=nrows,
        half=half,
        nodes_by_core=nodes_by_core,
        idx_in=idx_in,
        chunks=chunks,
        zero_blocks=zero_blocks,
        hi_first_blocks=hi_first_blocks,
        s_tot=s_tot,
        disb=disb,
        kb=kb,
    )


def _fold_weights(inp):
    """Fold BN into conv weights/biases. Returns per-layer U [Din,192], biases."""
    f = np.float32
    Us, biases = [], []
    for li, (wname, bname, bn) in enumerate(
        [("conv1", None, "bn1"), ("conv2", None, "bn2"), ("conv3", None, "bn3")]
    ):
        W = np.asarray(inp[wname + "_W"], dtype=f)  # [3, 60, Din]
        b = np.asarray(inp[wname + "_b"], dtype=f)  # [3, 60]
        g = np.asarray(inp[bn + "_g"], dtype=f)
        bb = np.asarray(inp[bn + "_b"], dtype=f)
        m = np.asarray(inp[bn + "_m"], dtype=f)
        v = np.asarray(inp[bn + "_v"], dtype=f)
        s = g / np.sqrt(v + EPS)
        t = bb - m * s
        s3 = s.reshape(3, H)
        t3 = t.reshape(3, H)
        Din = W.shape[2]
        U = np.zeros((Din, 192), dtype=f)
        U[:, 0:H] = (W[1] * s3[1][:, None]).T
        U[:, 64 : 64 + H] = (W[2] * s3[2][:, None]).T
        U[:, 128 : 128 + H] = (W[0] * s3[0][:, None]).T
        c0 = b[0] * s3[0] + t3[0]
        c1 = b[1] * s3[1] + t3[1]
        c2 = b[2] * s3[2] + t3[2]
        Us.append(U)
        biases.append((c0, c1, c2))
    return Us, biases


# ================================================================ device build
def _build_nc(plan):
    import concourse.bass as bass
    import concourse.bacc as bacc
    import concourse.mybir as mybir
    import concourse.tile as tile
    from concourse.masks import make_identity

    f32 = mybir.dt.float32
    i16 = mybir.dt.int16
    nblk = plan["nblk"]
    lpc = plan["lpc"]
    nrows = plan["nrows"]
    half = plan["half"]
    s_tot = plan["s_tot"]
    chunks = plan["chunks"]
    AX = mybir.AxisListType.X
    ADD = mybir.AluOpType.add
    MUL = mybir.AluOpType.mult

    nc = bacc.Bacc("TRN2", target_bir_lowering=False, debug=False, num_devices=NC,
                   num_swdge_queues=4, dynamic_dma_scratch_size=32768)

    # ---- I/O
    xT = nc.declare_dram_parameter("xT", [128, lpc], f32, isOutput=False)
    idx_d = nc.declare_dram_parameter("idx", [128, s_tot // 16], i16, isOutput=False)
    disb_d = nc.declare_dram_parameter("disb", [128, nblk], f32, isOutput=False)
    dis2b_d = nc.declare_dram_parameter("dis2b", [128, nblk], f32, isOutput=False)
    u_d = [
        nc.declare_dram_parameter("u0", [128, 192], f32, isOutput=False),
        (
            nc.declare_dram_parameter("u1a", [128, 192], f32, isOutput=False),
            nc.declare_dram_parameter("u1b", [64, 192], f32, isOutput=False),
        ),
        (
            nc.declare_dram_parameter("u2a", [128, 192], f32, isOutput=False),
            nc.declare_dram_parameter("u2b", [64, 192], f32, isOutput=False),
        ),
    ]
    bias_d = [
        [
            nc.declare_dram_parameter(f"bias{l}_{k}", [128, 64], f32, isOutput=False)
            for k in range(3)
        ]
        for l in range(3)
    ]
    lwa_d = nc.declare_dram_parameter("lwa", [128, N_CLS], f32, isOutput=False)
    lwb_d = nc.declare_dram_parameter("lwb", [64, N_CLS], f32, isOutput=False)
    blin_d = nc.declare_dram_parameter("blin", [128, N_CLS], f32, isOutput=False)
    out_d = nc.declare_dram_parameter("out", [lpc, N_CLS], f32, isOutput=True)

    # ---- internal DRAM (per layer to keep collective buffers single-writer)
    ystage = [nc.dram_tensor(f"ystage{l}", [lpc, 128], f32) for l in range(3)]
    yt = [
        nc.dram_tensor(f"yt{l}", [nrows, 128], f32, addr_space="Shared")
        for l in range(3)
    ]
    zstage = [nc.dram_tensor(f"zstage{l}", [lpc, 64], f32) for l in range(3)]
    zt = [
        nc.dram_tensor(f"zt{l}", [nrows, 64], f32, addr_space="Shared")
        for l in range(3)
    ]
    rg = [list(range(NC))]

    with tile.TileContext(nc) as tc, \
            tc.tile_pool(name="const", bufs=1) as const, \
            tc.tile_pool(name="big", bufs=1) as big:
        # persistent SBUF state
        XTa = big.tile([128, lpc], f32, tag="XTa")
        XTb = big.tile([64, lpc], f32, tag="XTb")
        T12 = big.tile([128, nblk * 128], f32, tag="T12")
        Y0 = big.tile([128, nblk * 64], f32, tag="Y0")
        R1 = big.tile([128, nblk * 128], f32, tag="R1")
        R2 = big.tile([128, nblk * 64], f32, tag="R2")
        ZL = big.tile([128, nblk * 64], f32, tag="ZL")
        OUTALL = big.tile([128, nblk * N_CLS], f32, tag="OUTALL")
        idx_sb = const.tile([128, s_tot // 16], i16)
        disb_sb = const.tile([128, nblk], f32)
        dis2b_sb = const.tile([128, nblk], f32)
        ident = const.tile([128, 128], f32)
        make_identity(nc, ident)

        nc.sync.dma_start(out=idx_sb[:], in_=idx_d[:])
        nc.sync.dma_start(out=disb_sb[:], in_=disb_d[:])
        nc.sync.dma_start(out=dis2b_sb[:], in_=dis2b_d[:])
        nc.sync.dma_start(out=XTa[:], in_=xT[:])

        u_sb = []
        u0 = const.tile([128, 192], f32)
        nc.sync.dma_start(out=u0[:], in_=u_d[0][:])
        u_sb.append((u0, None))
        for l in (1, 2):
            ua = const.tile([128, 192], f32, tag=f"u{l}a")
            ub = const.tile([64, 192], f32, tag=f"u{l}b")
            nc.sync.dma_start(out=ua[:], in_=u_d[l][0][:])
            nc.sync.dma_start(out=ub[:], in_=u_d[l][1][:])
            u_sb.append((ua, ub))
        bias_sb = []
        for l in range(3):
            row = []
            for k in range(3):
                t = const.tile([128, 64], f32, tag=f"b{l}{k}")
                nc.sync.dma_start(out=t[:], in_=bias_d[l][k][:])
                row.append(t)
            bias_sb.append(row)
        lwa = const.tile([128, N_CLS], f32)
        lwb = const.tile([64, N_CLS], f32)
        blin = const.tile([128, N_CLS], f32)
        nc.sync.dma_start(out=lwa[:], in_=lwa_d[:])
        nc.sync.dma_start(out=lwb[:], in_=lwb_d[:])
        nc.sync.dma_start(out=blin[:], in_=blin_d[:])

        # zero-init accumulators (pad-only blocks rely on this)
        nc.any.memset(R1[:], 0.0)
        nc.any.memset(R2[:], 0.0)
        nc.any.memset(ZL[:], 0.0)

        with (
            tc.tile_pool(name="psum", bufs=2, space="PSUM") as psum,
            tc.tile_pool(name="tpsum", bufs=2, space="PSUM") as tpsum,
            tc.tile_pool(name="g1p", bufs=4) as g1p,
            tc.tile_pool(name="g2p", bufs=4) as g2p,
            tc.tile_pool(name="work", bufs=4) as work,
        ):
            for l in range(3):
                ua, ub = u_sb[l]
                # ---------------- A: dense Y = X @ U ; split T12 / Y0
                for rb in range(nblk):
                    ps = psum.tile([128, 192], f32, tag="ps")
                    sl = slice(rb * 128, (rb + 1) * 128)
                    use_b = ub is not None and STAGE >= 6
                    nc.tensor.matmul(
                        out=ps[:],
                        lhsT=XTa[:, sl],
                        rhs=ua[:],
                        start=True,
                        stop=not use_b,
                    )
                    if use_b:
                        nc.tensor.matmul(
                            out=ps[:], lhsT=XTb[:, sl], rhs=ub[:],
                            start=False, stop=True,
                        )
                    nc.any.tensor_copy(out=T12[:, sl], in_=ps[:, 0:128])
                    nc.any.tensor_copy(
                        out=Y0[:, rb * 64 : (rb + 1) * 64], in_=ps[:, 128:192]
                    )
                # T12 *= dis (per row)  [128, nblk, 128] * [128, nblk, 1]
                nc.vector.tensor_tensor(
                    out=T12[:].rearrange("p (b e) -> p b e", e=128),
                    in0=T12[:].rearrange("p (b e) -> p b e", e=128),
                    in1=disb_sb[:].unsqueeze(2).to_broadcast([128, nblk, 128]),
                    op=MUL,
                )
                if STAGE < 2:
                    continue
                nc.sync.dma_start(
                    out=ystage[l][:].rearrange("(b p) e -> p b e", p=128),
                    in_=T12[:].rearrange("p (b e) -> p b e", e=128),
                )
                nc.gpsimd.collective_compute(
                    "AllGather", mybir.AluOpType.bypass,
                    replica_groups=rg, ins=[ystage[l][:]], outs=[yt[l][:]],
                )
                # per-layer zero of never-written blocks
                for b in plan["zero_blocks"]:
                    nc.any.memset(R1[:, b * 128 : (b + 1) * 128], 0.0)
                    nc.any.memset(R2[:, b * 64 : (b + 1) * 64], 0.0)

                # ---------------- B: SpMM1 (gather YT, reduce per block)
                if STAGE < 3:
                    continue
                for chi, ch in enumerate(chunks):
                    ncols = ch["ncols"]
                    slot0 = (0 if ch["p"] == 0 else plan["kb"][0].sum() * 128) + ch[
                        "col0"
                    ] * 128
                    gt = g1p.tile([128, CH_COLS, 128], f32, tag="g1")
                    src = yt[l][0:half, :] if ch["p"] == 0 else yt[l][half:nrows, :]
                    nc.gpsimd.dma_gather(
                        out_ap=gt[:, :ncols, :],
                        in_ap=src,
                        idxs_ap=idx_sb[:, slot0 // 16 : (slot0 + ncols * 128) // 16],
                        num_idxs=ncols * 128,
                        num_idxs_reg=ncols * 128,
                        elem_size=128,
                        single_packet=False,
                        queue_num=chi % 4,
                    )
                    for (b, coff, w, first) in ch["pieces"]:
                        first_write = first and (
                            ch["p"] == 0 or b in plan["hi_first_blocks"]
                        )
                        view = gt[:, coff : coff + w, :].rearrange("p c e -> p e c")
                        bsl = slice(b * 128, (b + 1) * 128)
                        if first_write:
                            nc.vector.tensor_reduce(
                                out=R1[:, bsl], in_=view, axis=AX, op=ADD
                            )
                        else:
                            tmp = work.tile([128, 128], f32, tag="t1")
                            nc.vector.tensor_reduce(
                                out=tmp[:], in_=view, axis=AX, op=ADD
                            )
                            nc.any.tensor_tensor(
                                out=R1[:, bsl], in0=R1[:, bsl], in1=tmp[:], op=ADD
                            )
                # self loop: R1 += T12
                if STAGE < 4:
                    continue
                nc.vector.tensor_tensor(out=R1[:], in0=R1[:], in1=T12[:], op=ADD)
                # Z2' = dis^2 * R1[:, 64:124] -> ZL (cols 0:60 of each 64-block)
                r1v = R1[:].rearrange("p (b e) -> p b e", e=128)
                zlv = ZL[:].rearrange("p (b e) -> p b e", e=64)
                nc.vector.tensor_tensor(
                    out=zlv[:, :, 0:60],
                    in0=r1v[:, :, 64:124],
                    in1=dis2b_sb[:].unsqueeze(2).to_broadcast([128, nblk, 60]),
                    op=MUL,
                )
                nc.sync.dma_start(
                    out=zstage[l][:].rearrange("(b p) e -> p b e", p=128),
                    in_=zlv,
                )
                nc.gpsimd.collective_compute(
                    "AllGather", mybir.AluOpType.bypass,
                    replica_groups=rg, ins=[zstage[l][:]], outs=[zt[l][:]],
                )

                # ---------------- C: SpMM2 (gather ZT, reduce)
                if STAGE < 5:
                    continue
                for chi, ch in enumerate(chunks):
                    ncols = ch["ncols"]
                    slot0 = (0 if ch["p"] == 0 else plan["kb"][0].sum() * 128) + ch[
                        "col0"
                    ] * 128
                    gt = g2p.tile([128, CH_COLS, 64], f32, tag="g2")
                    src = zt[l][0:half, :] if ch["p"] == 0 else zt[l][half:nrows, :]
                    nc.gpsimd.dma_gather(
                        out_ap=gt[:, :ncols, :],
                        in_ap=src,
                        idxs_ap=idx_sb[:, slot0 // 16 : (slot0 + ncols * 128) // 16],
                        num_idxs=ncols * 128,
                        num_idxs_reg=ncols * 128,
                        elem_size=64,
                        single_packet=False,
                        queue_num=chi % 4,
                    )
                    for (b, coff, w, first) in ch["pieces"]:
                        first_write = first and (
                            ch["p"] == 0 or b in plan["hi_first_blocks"]
                        )
                        view = gt[:, coff : coff + w, :].rearrange("p c e -> p e c")
                        bsl = slice(b * 64, (b + 1) * 64)
                        if first_write:
                            nc.vector.tensor_reduce(
                                out=R2[:, bsl], in_=view, axis=AX, op=ADD
                            )
                        else:
                            tmp = work.tile([128, 64], f32, tag="t2")
                            nc.vector.tensor_reduce(
                                out=tmp[:], in_=view, axis=AX, op=ADD
                            )
                            nc.any.tensor_tensor(
                                out=R2[:, bsl], in0=R2[:, bsl], in1=tmp[:], op=ADD
                            )
                # self loop: R2 += ZL
                nc.vector.tensor_tensor(out=R2[:], in0=R2[:], in1=ZL[:], op=ADD)

                # ---------------- D: assemble X' and transpose to X_T
                if STAGE < 6:
                    continue
                b0, b1, b2 = bias_sb[l]
                r2v = R2[:].rearrange("p (b e) -> p b e", e=64)
                y0v = Y0[:].rearrange("p (b e) -> p b e", e=64)
                for rb in range(nblk):
                    xn = work.tile([128, 192], f32, tag="xn")
                    nc.any.memset(xn[:, 180:192], 0.0)
                    # P0
                    nc.any.tensor_tensor(
                        out=xn[:, 0:60], in0=y0v[:, rb, 0:60],
                        in1=b0[:, 0:60], op=ADD,
                    )
                    # P1 = dis * R1 + c1
                    nc.any.tensor_scalar(
                        out=xn[:, 60:120],
                        in0=r1v[:, rb, 0:60],
                        scalar1=disb_sb[:, rb : rb + 1],
                        scalar2=None,
                        op0=MUL,
                    )
                    nc.any.tensor_tensor(
                        out=xn[:, 60:120], in0=xn[:, 60:120],
                        in1=b1[:, 0:60], op=ADD,
                    )
                    # P2 = dis * R2 + c2
                    nc.any.tensor_scalar(
                        out=xn[:, 120:180],
                        in0=r2v[:, rb, 0:60],
                        scalar1=disb_sb[:, rb : rb + 1],
                        scalar2=None,
                        op0=MUL,
                    )
                    nc.any.tensor_tensor(
                        out=xn[:, 120:180], in0=xn[:, 120:180],
                        in1=b2[:, 0:60], op=ADD,
                    )
                    sl = slice(rb * 128, (rb + 1) * 128)
                    pt = tpsum.tile([128, 128], f32, tag="pt")
                    nc.tensor.transpose(out=pt[:], in_=xn[:, 0:128], identity=ident[:])
                    nc.any.tensor_copy(out=XTa[:, sl], in_=pt[:])
                    pt2 = tpsum.tile([64, 128], f32, tag="pt2")
                    nc.tensor.transpose(
                        out=pt2[:], in_=xn[:, 128:192], identity=ident[:]
                    )
                    nc.any.tensor_copy(out=XTb[:, sl], in_=pt2[:])

            # ---------------- final linear
            for rb in range(nblk):
                sl = slice(rb * 128, (rb + 1) * 128)
                ps = psum.tile([128, N_CLS], f32, tag="pf")
                nc.tensor.matmul(
                    out=ps[:], lhsT=XTa[:, sl], rhs=lwa[:], start=True,
                    stop=STAGE < 6,
                )
                if STAGE >= 6:
                    nc.tensor.matmul(
                        out=ps[:], lhsT=XTb[:, sl], rhs=lwb[:], start=False, stop=True
                    )
                nc.any.tensor_tensor(
                    out=OUTALL[:, rb * N_CLS : (rb + 1) * N_CLS],
                    in0=ps[:], in1=blin[:], op=ADD,
                )
            nc.sync.dma_start(
                out=out_d[:].rearrange("(b p) c -> p b c", p=128),
                in_=OUTALL[:].rearrange("p (b c) -> p b c", c=N_CLS),
            )

    nc.compile()
    return nc


# ================================================================ entry point
def _prepare_inputs(inputs, plan):
    """Build per-core in_maps."""
    f = np.float32
    x = np.asarray(inputs["x"], dtype=f)
    nbc = plan["nodes_by_core"]
    lpc, lpc_real, nblk = plan["lpc"], plan["lpc_real"], plan["nblk"]
    Us, biases = _fold_weights(inputs)
    lin_W = np.asarray(inputs["lin_W"], dtype=f)  # [40, 180]
    lin_b = np.asarray(inputs["lin_b"], dtype=f)
    lwT = np.zeros((192, N_CLS), dtype=f)
    lwT[0:180, :] = lin_W.T
    blin = np.tile(lin_b[None, :], (128, 1)).astype(f)

    def repl_bias(c):
        t = np.zeros((128, 64), dtype=f)
        t[:, 0:60] = c[None, :]
        return t

    in_maps = []
    for c in range(NC):
        m = {}
        xs = np.zeros((lpc, D_IN), dtype=f)
        xs[:lpc_real] = x[nbc[c]]
        m["xT"] = np.ascontiguousarray(xs.T)
        m["idx"] = plan["idx_in"][c]
        m["disb"] = np.ascontiguousarray(plan["disb"][c])
        m["dis2b"] = np.ascontiguousarray(plan["disb"][c] ** 2)
        m["u0"] = Us[0]
        m["u1a"] = np.ascontiguousarray(Us[1][0:128])
        u1b = np.zeros((64, 192), dtype=f)
        u1b[0:52] = Us[1][128:180]
        m["u1b"] = u1b
        m["u2a"] = np.ascontiguousarray(Us[2][0:128])
        u2b = np.zeros((64, 192), dtype=f)
        u2b[0:52] = Us[2][128:180]
        m["u2b"] = u2b
        for l in range(3):
            for k in range(3):
                m[f"bias{l}_{k}"] = repl_bias(biases[l][k])
        m["lwa"] = np.ascontiguousarray(lwT[0:128])
        m["lwb"] = np.ascontiguousarray(lwT[128:192])
        m["blin"] = blin
        in_maps.append(m)
    return in_maps


_CACHE = {}


def _install_ntff_shim():
    """Provide antenv.axon_hooks (missing in this image) so trace=True works."""
    import sys, types, ctypes, contextlib

    try:
        from antenv.axon_hooks import get_axon_ntff_profile_hook  # noqa: F401

        return
    except ImportError:
        pass
    so_path = "/opt/axon/libaxon_pjrt.so"
    hook = None
    try:
        lib = ctypes.CDLL(so_path)
        if hasattr(lib, "axon_start_nrt_profile"):
            lib.axon_start_nrt_profile.argtypes = [
                ctypes.POINTER(ctypes.c_int64),
                ctypes.c_size_t,
            ]
            lib.axon_start_nrt_profile.restype = ctypes.c_int64
            lib.axon_stop_nrt_profile.argtypes = [ctypes.c_char_p]
            lib.axon_stop_nrt_profile.restype = ctypes.c_int64

            @contextlib.contextmanager
            def hook(output_dir, device_ids):
                import jax

                jax.devices()
                if device_ids:
                    ids = (ctypes.c_int64 * len(device_ids))(*device_ids)
                    rc = lib.axon_start_nrt_profile(ids, len(device_ids))
                else:
                    rc = lib.axon_start_nrt_profile(None, 0)
                if rc != 0:
                    raise RuntimeError(f"axon_start_nrt_profile rc={rc}")
                try:
                    yield
                finally:
                    n = lib.axon_stop_nrt_profile(str(output_dir).encode())
                    print(f"profile: {n} file(s) written to {output_dir}")

    except OSError:
        pass
    mod = types.ModuleType("antenv.axon_hooks")
    mod.get_axon_ntff_profile_hook = lambda: hook
    mod.set_axon_ntff_profile_hook = lambda h: None
    sys.modules["antenv.axon_hooks"] = mod


def kernel(**inputs):
    global LAST_EXEC_NS, LAST_PROFILE
    from concourse import bass_utils

    if TRACE:
        _install_ntff_shim()
        bass_utils.upload_artifacts = lambda tmpdir: tmpdir

    edge_index = np.asarray(inputs["edge_index"])
    key = ("plan", edge_index.shape[1])
    if key not in _CACHE:
        plan = _make_plan(edge_index, N_NODES)
        nc = _build_nc(plan)
        _CACHE[key] = (plan, nc)
    plan, nc = _CACHE[key]

    in_maps = _prepare_inputs(inputs, plan)
    if os.environ.get("MIXHOP_SIM"):
        from concourse import bass_interp

        sim = bass_interp.MultiCoreSim(nc, NC, num_workers=NC)
        for c in range(NC):
            for k, v in in_maps[c].items():
                sim.cores[c].tensor(k)[:] = v
        sim.simulate()
        outs = [{"out": np.array(sim.cores[c].mem_tensor("out"))} for c in range(NC)]
    else:
        res = bass_utils.run_bass_kernel_spmd(
            nc, in_maps, core_ids=list(range(NC)), trace=TRACE
        )
        LAST_EXEC_NS = res.exec_time_ns
        LAST_PROFILE = res.profile_json
        outs = res.results

    lpc, lpc_real = plan["lpc"], plan["lpc_real"]
    nbc = plan["nodes_by_core"]
    full = np.empty((N_NODES, N_CLS), dtype=np.float32)
    for c in range(NC):
        full[nbc[c]] = outs[c]["out"][:lpc_real]
    return full
